# revision 19
# baseline (speedup 1.0000x reference)
"""Multi-head attention Trainium2 kernel (nn_MultiHeadAttention_7035156430929).

B=4, S=1024, E=1024, H=16, D=64. Sharding: 8 cores = 4 batches x 2
head-groups (tensor parallel over heads, per the hint). Each core computes 8
heads for all 1024 queries/keys of its batch: Wq/Wk/Wv column-sliced (512
features), Wo row-sliced, giving a PARTIAL output [1024, 1024] per core; the
two partials of a batch are summed on the host (the "all-reduce after fc_out"
done host-side since core outputs are gathered anyway).

Everything on device is bf16 except PSUM accumulation, softmax denominators,
biases, and the output partials (fp32). Measured-HW design points:
  - matmul N=512 ~285 ns; K=64 pairs on PE row-groups (tile_position 0/64)
    run concurrently (~308 ns/pair) -> energy QK^T paired across head pairs.
  - exp on ACT: [128,1024] = 1324 ns; energy PSUM tiles are [128,1024]
    (2 banks) so each (head, kt) needs ONE activation.
  - PE emission is hand-woven: energy groups are spaced between projection /
    PV / output chains so the ACT engine (85 us of exp) hides under the PE
    stream (~128 us) and energy PSUM slots (3x2 banks) never block.
  - In the For_i timing loop (whose per-iteration all-engine barrier
    serializes bodies), input tiles persist and each iteration RE-LOADS them
    at its tail, so the HBM traffic stays per-iteration but the next body's
    projections start right after the barrier (no DMA lead-in).
1/sqrt(E) folds into Wq/bq; bv folds into the g=0 core's output bias
(softmax rows sum to 1); key-padding mask becomes an additive per-key bias
(-50) inside the exp activation.
"""
import sys

sys.path.insert(0, "/opt/trn_rl_repo")

from contextlib import ExitStack

import numpy as np
import ml_dtypes

import concourse.bacc as bacc
import concourse.tile as tile
from concourse import mybir
from concourse.bass_utils import run_bass_kernel_spmd

B, S, E, H, D = 4, 1024, 1024, 16, 64
P = 128
N_CORES = 8
HC = 8             # heads per core
EH = 512           # features per core
FT = 4             # feature tiles of 128 (2 heads each)
ET = 8             # contraction tiles over E
NKT = 8            # key-token tiles
DP1 = D + 1        # V columns per head incl. ones column
F32 = mybir.dt.float32
BF16 = mybir.dt.bfloat16
AF = mybir.ActivationFunctionType
NPBF16 = ml_dtypes.bfloat16


def _declare(nc):
    dp = nc.declare_dram_parameter
    t = {}
    t["xqT"] = dp("xqT", [E, S], BF16, isOutput=False)   # query[b].T
    t["xkT"] = dp("xkT", [E, S], BF16, isOutput=False)
    t["xvT"] = dp("xvT", [E, S], BF16, isOutput=False)
    t["wq"] = dp("wq", [E, EH], BF16, isOutput=False)    # Wq.T col-slice, *s
    t["wk"] = dp("wk", [E, EH], BF16, isOutput=False)
    t["wv"] = dp("wv", [E, EH], BF16, isOutput=False)
    t["wo"] = dp("wo", [EH, E], BF16, isOutput=False)    # Wo.T row-slice
    t["bq"] = dp("bq", [P, FT], F32, isOutput=False)
    t["bk"] = dp("bk", [P, FT], F32, isOutput=False)
    t["maskb"] = dp("maskb", [P, NKT], F32, isOutput=False)  # 0 or -50
    t["boeff"] = dp("boeff", [P, E], BF16, isOutput=False)   # row-replicated
    t["ones"] = dp("ones", [P, D], BF16, isOutput=False)
    t["out"] = dp("out", [S, E], F32, isOutput=True)         # partial
    t["out2"] = dp("out2", [S, E], F32, isOutput=True)       # timing-loop pair
    return t


def _emit_prelude(nc, tc, t):
    """Create all pools/tiles and emit the initial input loads. Returns a
    state dict shared by every _emit_compute call (tiles persist across
    For_i iterations; each iteration refills them at its tail so the next
    iteration's projections start immediately after the loop barrier)."""
    xqT, xkT, xvT = t["xqT"], t["xkT"], t["xvT"]
    wq, wk, wv, wo = t["wq"], t["wk"], t["wv"], t["wo"]
    bq, bk, maskb, boeff = t["bq"], t["bk"], t["maskb"], t["boeff"]
    ones = t["ones"]

    ctx = ExitStack()
    const = ctx.enter_context(tc.tile_pool(name="const", bufs=1))
    bq_t = const.tile([P, FT], F32, tag="bq", name="bq")
    bk_t = const.tile([P, FT], F32, tag="bk", name="bk")
    mb_t = const.tile([P, NKT], F32, tag="mb", name="mb")
    bo_t = const.tile([P, E], BF16, tag="bo", name="bo")
    on_t = const.tile([1, D], BF16, tag="on", name="on")
    nc.gpsimd.dma_start(bq_t[:], bq.ap()[:])
    nc.gpsimd.dma_start(bk_t[:], bk.ap()[:])
    nc.gpsimd.dma_start(mb_t[:], maskb.ap()[:])
    nc.gpsimd.dma_start(bo_t[:], boeff.ap()[:])
    nc.gpsimd.dma_start(on_t[:], ones.ap()[0:1, :])

    # Persistent per-iteration intermediates
    kt_pool = ctx.enter_context(tc.tile_pool(name="ktp", bufs=FT))
    qt_pool = ctx.enter_context(tc.tile_pool(name="qtp", bufs=FT))
    va_pool = ctx.enter_context(tc.tile_pool(name="vap", bufs=NKT))
    atn_pool = ctx.enter_context(tc.tile_pool(name="atn", bufs=FT))
    pt_pool = ctx.enter_context(tc.tile_pool(name="pt", bufs=38))
    nrm_pool = ctx.enter_context(tc.tile_pool(name="nrm", bufs=2))
    ob_pool = ctx.enter_context(tc.tile_pool(name="ob", bufs=2))
    wo_pool = ctx.enter_context(tc.tile_pool(name="wop", bufs=1))
    ps2 = ctx.enter_context(tc.tile_pool(name="ps2", bufs=3, space="PSUM"))
    ps1 = ctx.enter_context(tc.tile_pool(name="ps1", bufs=2, space="PSUM"))

    KT = [kt_pool.tile([P, S], BF16, tag="kt", name="kt") for _ in range(FT)]
    QT = [qt_pool.tile([P, S], BF16, tag="qt", name="qt") for _ in range(FT)]
    VA = [va_pool.tile([P, HC * DP1], BF16, tag="va", name="va")
          for _ in range(NKT)]
    ATN = [atn_pool.tile([P, S], BF16, tag="at", name="at") for _ in range(FT)]
    PT = {}

    # Streamed activations / weights (per-et tiles; first chain starts after
    # just its first two small DMAs land)
    xk_p = ctx.enter_context(tc.tile_pool(name="xk", bufs=ET))
    xq_p = ctx.enter_context(tc.tile_pool(name="xq", bufs=ET))
    wk_p = ctx.enter_context(tc.tile_pool(name="wk", bufs=FT))
    wq_p = ctx.enter_context(tc.tile_pool(name="wq", bufs=FT))
    xv_p = ctx.enter_context(tc.tile_pool(name="xv", bufs=ET))
    wv_p = ctx.enter_context(tc.tile_pool(name="wv", bufs=ET))

    xk_t = [xk_p.tile([P, S], BF16, tag="xk", name="xk") for _ in range(ET)]
    xq_t = [xq_p.tile([P, S], BF16, tag="xq", name="xq") for _ in range(ET)]
    wk_t = [wk_p.tile([P, ET * P], BF16, tag="wk", name="wk")
            for _ in range(FT)]
    wq_t = [wq_p.tile([P, ET * P], BF16, tag="wq", name="wq")
            for _ in range(FT)]
    xv_t = [xv_p.tile([P, S], BF16, tag="xv", name="xv") for _ in range(ET)]
    wv_t = [wv_p.tile([P, EH], BF16, tag="wv", name="wv") for _ in range(ET)]
    wo_t = wo_pool.tile([P, FT * E], BF16, tag="wo", name="wo")

    def _wf_load(tt, dram, ft):
        nc.sync.dma_start(
            tt[:].rearrange("p (a s) -> p a s", s=P),
            dram.ap()[:, ft * P:(ft + 1) * P].rearrange(
                "(a p) s -> p a s", p=P))

    def emit_loads():
        for et in range(ET):
            nc.sync.dma_start(xk_t[et][:], xkT.ap()[et * P:(et + 1) * P, :])
        _wf_load(wk_t[0], wk, 0)
        for et in range(ET):
            nc.sync.dma_start(xq_t[et][:], xqT.ap()[et * P:(et + 1) * P, :])
        _wf_load(wq_t[0], wq, 0)
        for ft in range(1, FT):
            _wf_load(wk_t[ft], wk, ft)
            _wf_load(wq_t[ft], wq, ft)
        for et in range(ET):
            nc.sync.dma_start(xv_t[et][:], xvT.ap()[et * P:(et + 1) * P, :])
            nc.sync.dma_start(wv_t[et][:], wv.ap()[et * P:(et + 1) * P, :])
        nc.sync.dma_start(
            wo_t[:].rearrange("p (a s) -> p a s", s=E),
            wo.ap()[:].rearrange("(a p) s -> p a s", p=P))

    emit_loads()
    for tt in range(NKT):
        ones3 = VA[tt][:].rearrange("p (h c) -> p h c", c=DP1)[:, :, D:DP1]
        nc.gpsimd.dma_start(
            ones3, ones.ap()[:, 0:HC].rearrange("p (h c) -> p h c", c=1))

    return dict(ctx=ctx, KT=KT, QT=QT, VA=VA, ATN=ATN, on_t=on_t,
                kt_b=(bq_t, bk_t, mb_t, bo_t),
                xk_t=xk_t, xq_t=xq_t, wk_t=wk_t, wq_t=wq_t,
                xv_t=xv_t, wv_t=wv_t, wo_t=wo_t,
                ps2=ps2, ps1=ps1, pt_pool=pt_pool, nrm_pool=nrm_pool,
                ob_pool=ob_pool, emit_loads=emit_loads)


def _emit_compute(nc, tc, t, h, out_key="out", refill=False,
                  emit_out=True, emit_pv3=True):
    out = t[out_key]
    KT, QT, VA, ATN = h["KT"], h["QT"], h["VA"], h["ATN"]
    bq_t, bk_t, mb_t, bo_t = h["kt_b"]
    xk_t, xq_t, wk_t, wq_t = h["xk_t"], h["xq_t"], h["wk_t"], h["wq_t"]
    xv_t, wv_t, wo_t = h["xv_t"], h["wv_t"], h["wo_t"]
    ps2, ps1 = h["ps2"], h["ps1"]
    pt_pool, nrm_pool, ob_pool = h["pt_pool"], h["nrm_pool"], h["ob_pool"]
    PT = {}

    proj_ps = {}

    def proj_half(ft, kh, w_t, x_t, dst, bias_t):
        # half-chain unit (8 mm); evict with the second half
        if kh == 0:
            proj_ps[id(dst) ^ ft] = ps2.tile([P, S], F32, tag="p2", name="p2")
        ps = proj_ps[id(dst) ^ ft]
        for et in range(ET):
            nc.tensor.matmul(
                ps[:, kh * 512:(kh + 1) * 512],
                w_t[ft][:, et * P:(et + 1) * P],
                x_t[et][:, kh * 512:(kh + 1) * 512],
                start=(et == 0), stop=(et == ET - 1))
        if kh == 1:
            nc.vector.tensor_scalar_add(dst[ft][:], ps[:],
                                        bias_t[:, ft:ft + 1])

    def vproj(tt):
        ps = ps1.tile([P, 512], F32, tag="p1", name="p1")
        for et in range(ET):
            nc.tensor.matmul(
                ps[:], xv_t[et][:, tt * P:(tt + 1) * P], wv_t[et][:],
                start=(et == 0), stop=(et == ET - 1))
        va3 = VA[tt][:].rearrange("p (h c) -> p h c", c=DP1)[:, :, 0:D]
        ps3 = ps[:].rearrange("p (h c) -> p h c", c=D)
        nc.vector.tensor_copy(va3, ps3)

    def energy(ft, kt):
        pe_h = [ps2.tile([P, S], F32, tag="p2", name="p2") for _ in range(2)]
        for qh in range(2):
            for hh in range(2):
                hp = hh * D
                nc.tensor.matmul(
                    pe_h[hh][:, qh * 512:(qh + 1) * 512],
                    KT[ft][hp:hp + D, kt * P:(kt + 1) * P],
                    QT[ft][hp:hp + D, qh * 512:(qh + 1) * 512],
                    start=True, stop=True, tile_position=(hp, 0))
        for hh in range(2):
            pt = pt_pool.tile([P, S], BF16, tag="pt", name="pt")
            nc.scalar.activation(pt[:], pe_h[hh][:], AF.Exp,
                                 bias=mb_t[:, kt:kt + 1])
            PT[(ft, hh, kt)] = pt

    pv_live = {}

    def pv_mm(ft, hh, qh):
        # PV accumulation + reciprocal; the broadcast+multiply finisher is
        # deferred (pv_fin) so its Pool/DVE latency stays off the window
        # critical path.
        hc = ft * 2 + hh
        ps = ps1.tile([DP1, 512], F32, tag="p1", name="p1")
        for kt in range(NKT):
            nc.tensor.matmul(
                ps[:], VA[kt][:, hc * DP1:(hc + 1) * DP1],
                PT[(ft, hh, kt)][:, qh * 512:(qh + 1) * 512],
                start=(kt == 0), stop=(kt == NKT - 1))
        rec = nrm_pool.tile([1, 512], F32, tag="rec", name="rec")
        nc.vector.reciprocal(rec[0:1, :], ps[D:DP1, :])
        pv_live[(ft, hh, qh)] = (ps, rec)

    def pv_fin(ft, hh, qh):
        ps, rec = pv_live.pop((ft, hh, qh))
        bc = nrm_pool.tile([D, 512], F32, tag="bc", name="bc")
        nc.gpsimd.partition_broadcast(bc[0:D, :], rec[0:1, :])
        nc.vector.tensor_mul(
            ATN[ft][hh * D:(hh + 1) * D, qh * 512:(qh + 1) * 512],
            ps[0:D, :], bc[0:D, :])

    def pv(ft, hh, qh):
        pv_mm(ft, hh, qh)
        pv_fin(ft, hh, qh)

    def outp(qt):
        # Both 512-wide output halves accumulate into one [128,1024] psum
        # tile: 8 MMs, then a single DVE bias-add and a single DMA store.
        ps = ps2.tile([P, S], F32, tag="p2", name="p2")
        for ft in range(FT):
            for eb in range(2):
                nc.tensor.matmul(
                    ps[:, eb * 512:(eb + 1) * 512],
                    ATN[ft][:, qt * P:(qt + 1) * P],
                    wo_t[:, ft * E + eb * 512:ft * E + (eb + 1) * 512],
                    start=(ft == 0), stop=(ft == FT - 1))
        ob = ob_pool.tile([P, S], F32, tag="ob", name="ob")
        nc.vector.tensor_add(ob[:], ps[:], bo_t[:])
        nc.scalar.dma_start(out.ap()[qt * P:(qt + 1) * P, :], ob[:])

    # ---- hand-woven emission order ----
    # One ~2.3us filler unit after each energy group paces E emission at the
    # ACT exp drain rate (~2.7us/group). V runs in windows 0-1 so pv(ft)
    # lands at window ft+2 (PT in flight <= 34, pool 35). The tail orders
    # pv(3) by q-half so OUT chains for qt 0-3 overlap pv(3,*,1).
    def K(ft, kh):
        return lambda: proj_half(ft, kh, wk_t, xk_t, KT, bk_t)

    def Q(ft, kh):
        return lambda: proj_half(ft, kh, wq_t, xq_t, QT, bq_t)

    def seq(*fs):
        return lambda: [f() for f in fs]

    def MM(ft, hh, qh):
        return lambda: pv_mm(ft, hh, qh)

    def FIN(ft, hh, qh):
        return lambda: pv_fin(ft, hh, qh)

    for f in (K(0, 0), K(0, 1), Q(0, 0), Q(0, 1)):
        f()
    fillers = [
        K(1, 0), K(1, 1), Q(1, 0), Q(1, 1),
    ] + [lambda tt=tt: vproj(tt) for tt in range(NKT)] + [
        K(2, 0), K(2, 1), Q(2, 0), Q(2, 1),
        MM(0, 0, 0),
        seq(FIN(0, 0, 0), MM(0, 0, 1)),
        seq(FIN(0, 0, 1), MM(0, 1, 0)),
        seq(FIN(0, 1, 0), MM(0, 1, 1)),
        seq(FIN(0, 1, 1), K(3, 0)),
        K(3, 1), Q(3, 0), Q(3, 1),
        MM(1, 0, 0),
        seq(FIN(1, 0, 0), MM(1, 0, 1)),
        seq(FIN(1, 0, 1), MM(1, 1, 0)),
        seq(FIN(1, 1, 0), MM(1, 1, 1)),
        seq(FIN(1, 1, 1), MM(2, 0, 0)),
        seq(FIN(2, 0, 0), MM(2, 0, 1)),
        seq(FIN(2, 0, 1), MM(2, 1, 0)),
        seq(FIN(2, 1, 0), MM(2, 1, 1)),
    ]
    fi = iter(fillers)
    for ft in range(FT):
        for kt in range(NKT):
            energy(ft, kt)
            f = next(fi, None)
            if f is not None:
                f()
    pv_fin(2, 1, 1)
    if emit_pv3:
        pv_mm(3, 0, 0)
        pv_mm(3, 1, 0)
        pv_fin(3, 0, 0)
        pv_mm(3, 0, 1)
        pv_fin(3, 1, 0)
    if emit_out:
        outp(0)
        if emit_pv3:
            pv_mm(3, 1, 1)
        outp(1)
        if emit_pv3:
            pv_fin(3, 0, 1)
            pv_fin(3, 1, 1)
        for qt in range(2, ET):
            outp(qt)
    elif emit_pv3:
        pv_mm(3, 1, 1)
        pv_fin(3, 0, 1)
        pv_fin(3, 1, 1)
    if refill:
        h["emit_loads"]()


def build_nc(repeats=1, hw_loop=0, refill=True, emit_out=True, emit_pv3=True):
    nc = bacc.Bacc()
    t = _declare(nc)
    with tile.TileContext(nc) as tc:
        h = _emit_prelude(nc, tc, t)
        if hw_loop:
            with tc.For_i(0, hw_loop, 1):
                _emit_compute(nc, tc, t, h, refill=refill,
                              emit_out=emit_out, emit_pv3=emit_pv3)
        else:
            for _ in range(repeats):
                _emit_compute(nc, tc, t, h, refill=False)
        h["ctx"].close()
    nc.finalize()
    return nc


_NC = None


def _get_nc():
    global _NC
    if _NC is None:
        _NC = build_nc()
    return _NC


def _prep_in_maps(value, key_in, query, mask, Wq, bq, Wk, bk, Wv, bv, Wo, bo):
    f = np.float32
    value = np.asarray(value, f)
    key_in = np.asarray(key_in, f)
    query = np.asarray(query, f)
    mask = np.asarray(mask)
    Wq = np.asarray(Wq, f); bq = np.asarray(bq, f)
    Wk = np.asarray(Wk, f); bk = np.asarray(bk, f)
    Wv = np.asarray(Wv, f); bv = np.asarray(bv, f)
    Wo = np.asarray(Wo, f); bo = np.asarray(bo, f)

    s = f(1.0 / np.sqrt(E))
    wqT = (Wq.T * s).astype(NPBF16)
    wkT = Wk.T.astype(NPBF16)
    wvT = Wv.T.astype(NPBF16)
    woT = Wo.T.astype(NPBF16)
    bo_eff = bo + Wo @ bv
    bo_full = np.ascontiguousarray(
        np.broadcast_to(bo_eff, (P, E))).astype(NPBF16)
    bo_zero = np.zeros((P, E), NPBF16)
    ones_t = np.ones((P, D), NPBF16)

    xT = {}
    for b in range(B):
        xT[b] = (
            np.ascontiguousarray(query[b].astype(NPBF16).T),
            np.ascontiguousarray(key_in[b].astype(NPBF16).T),
            np.ascontiguousarray(value[b].astype(NPBF16).T),
        )

    in_maps = []
    for c in range(N_CORES):
        b, g = c // 2, c % 2
        cols = slice(g * EH, (g + 1) * EH)
        mrow = mask[b, 0, 0, :]
        mb = np.where(mrow == 0, f(-50.0), f(0.0)).astype(f)
        xq_b, xk_b, xv_b = xT[b]
        in_maps.append({
            "xqT": xq_b, "xkT": xk_b, "xvT": xv_b,
            "wq": np.ascontiguousarray(wqT[:, cols]),
            "wk": np.ascontiguousarray(wkT[:, cols]),
            "wv": np.ascontiguousarray(wvT[:, cols]),
            "wo": np.ascontiguousarray(woT[cols, :]),
            "bq": np.ascontiguousarray((bq[cols] * s).reshape(FT, P).T),
            "bk": np.ascontiguousarray(bk[cols].reshape(FT, P).T),
            "maskb": np.ascontiguousarray(mb.reshape(NKT, P).T),
            "boeff": bo_full if g == 0 else bo_zero,
            "ones": ones_t,
        })
    return in_maps


def _assemble(results):
    out = np.empty((B, S, E), np.float32)
    for b in range(B):
        out[b] = results[2 * b]["out"] + results[2 * b + 1]["out"]
    return out


def kernel(value, key_in, query, mask, Wq, bq, Wk, bk, Wv, bv, Wo, bo):
    nc = _get_nc()
    in_maps = _prep_in_maps(value, key_in, query, mask,
                            Wq, bq, Wk, bk, Wv, bv, Wo, bo)
    r = run_bass_kernel_spmd(nc, in_maps, list(range(N_CORES)))
    return _assemble(r.results)


def kernel_traced(value, key_in, query, mask, Wq, bq, Wk, bk, Wv, bv, Wo, bo,
                  **trace_kwargs):
    """Like kernel() but returns (output, BassKernelResults) with profiling."""
    nc = _get_nc()
    in_maps = _prep_in_maps(value, key_in, query, mask,
                            Wq, bq, Wk, bk, Wv, bv, Wo, bo)
    r = run_bass_kernel_spmd(nc, in_maps, list(range(N_CORES)), trace=True,
                             **trace_kwargs)
    return _assemble(r.results), r



# revision 20
# speedup vs baseline: 1.0011x; 1.0011x over previous
"""Multi-head attention Trainium2 kernel (nn_MultiHeadAttention_7035156430929).

B=4, S=1024, E=1024, H=16, D=64. Sharding: 8 cores = 4 batches x 2
head-groups (tensor parallel over heads, per the hint). Each core computes 8
heads for all 1024 queries/keys of its batch: Wq/Wk/Wv column-sliced (512
features), Wo row-sliced, giving a PARTIAL output [1024, 1024] per core; the
two partials of a batch are summed on the host (the "all-reduce after fc_out"
done host-side since core outputs are gathered anyway).

Everything on device is bf16 except PSUM accumulation, softmax denominators,
biases, and the output partials (fp32). Measured-HW design points:
  - matmul N=512 ~285 ns; K=64 pairs on PE row-groups (tile_position 0/64)
    run concurrently (~308 ns/pair) -> energy QK^T paired across head pairs.
  - exp on ACT: [128,1024] = 1324 ns; energy PSUM tiles are [128,1024]
    (2 banks) so each (head, kt) needs ONE activation.
  - PE emission is hand-woven: energy groups are spaced between projection /
    PV / output chains so the ACT engine (85 us of exp) hides under the PE
    stream (~128 us) and energy PSUM slots (3x2 banks) never block.
  - In the For_i timing loop (whose per-iteration all-engine barrier
    serializes bodies), input tiles persist and each iteration RE-LOADS them
    at its tail, so the HBM traffic stays per-iteration but the next body's
    projections start right after the barrier (no DMA lead-in).
1/sqrt(E) folds into Wq/bq; bv folds into the g=0 core's output bias
(softmax rows sum to 1); key-padding mask becomes an additive per-key bias
(-50) inside the exp activation.
"""
import sys

sys.path.insert(0, "/opt/trn_rl_repo")

from contextlib import ExitStack

import numpy as np
import ml_dtypes

import concourse.bacc as bacc
import concourse.tile as tile
from concourse import mybir
from concourse.bass_utils import run_bass_kernel_spmd

B, S, E, H, D = 4, 1024, 1024, 16, 64
P = 128
N_CORES = 8
HC = 8             # heads per core
EH = 512           # features per core
FT = 4             # feature tiles of 128 (2 heads each)
ET = 8             # contraction tiles over E
NKT = 8            # key-token tiles
DP1 = D + 1        # V columns per head incl. ones column
F32 = mybir.dt.float32
BF16 = mybir.dt.bfloat16
AF = mybir.ActivationFunctionType
NPBF16 = ml_dtypes.bfloat16


def _declare(nc):
    dp = nc.declare_dram_parameter
    t = {}
    t["xqT"] = dp("xqT", [E, S], BF16, isOutput=False)   # query[b].T
    t["xkT"] = dp("xkT", [E, S], BF16, isOutput=False)
    t["xvT"] = dp("xvT", [E, S], BF16, isOutput=False)
    t["wq"] = dp("wq", [E, EH], BF16, isOutput=False)    # Wq.T col-slice, *s
    t["wk"] = dp("wk", [E, EH], BF16, isOutput=False)
    t["wv"] = dp("wv", [E, EH], BF16, isOutput=False)
    t["wo"] = dp("wo", [EH, E], BF16, isOutput=False)    # Wo.T row-slice
    t["bq"] = dp("bq", [P, FT], F32, isOutput=False)
    t["bk"] = dp("bk", [P, FT], F32, isOutput=False)
    t["maskb"] = dp("maskb", [P, NKT], F32, isOutput=False)  # 0 or -50
    t["boeff"] = dp("boeff", [P, E], BF16, isOutput=False)   # row-replicated
    t["ones"] = dp("ones", [P, D], BF16, isOutput=False)
    t["out"] = dp("out", [S, E], F32, isOutput=True)         # partial
    t["out2"] = dp("out2", [S, E], F32, isOutput=True)       # timing-loop pair
    return t


def _emit_prelude(nc, tc, t):
    """Create all pools/tiles and emit the initial input loads. Returns a
    state dict shared by every _emit_compute call (tiles persist across
    For_i iterations; each iteration refills them at its tail so the next
    iteration's projections start immediately after the loop barrier)."""
    xqT, xkT, xvT = t["xqT"], t["xkT"], t["xvT"]
    wq, wk, wv, wo = t["wq"], t["wk"], t["wv"], t["wo"]
    bq, bk, maskb, boeff = t["bq"], t["bk"], t["maskb"], t["boeff"]
    ones = t["ones"]

    ctx = ExitStack()
    const = ctx.enter_context(tc.tile_pool(name="const", bufs=1))
    bq_t = const.tile([P, FT], F32, tag="bq", name="bq")
    bk_t = const.tile([P, FT], F32, tag="bk", name="bk")
    mb_t = const.tile([P, NKT], F32, tag="mb", name="mb")
    bo_t = const.tile([P, E], BF16, tag="bo", name="bo")
    on_t = const.tile([1, D], BF16, tag="on", name="on")
    nc.gpsimd.dma_start(bq_t[:], bq.ap()[:])
    nc.gpsimd.dma_start(bk_t[:], bk.ap()[:])
    nc.gpsimd.dma_start(mb_t[:], maskb.ap()[:])
    nc.gpsimd.dma_start(bo_t[:], boeff.ap()[:])
    nc.gpsimd.dma_start(on_t[:], ones.ap()[0:1, :])

    # Persistent per-iteration intermediates
    kt_pool = ctx.enter_context(tc.tile_pool(name="ktp", bufs=FT))
    qt_pool = ctx.enter_context(tc.tile_pool(name="qtp", bufs=FT))
    va_pool = ctx.enter_context(tc.tile_pool(name="vap", bufs=NKT))
    atn_pool = ctx.enter_context(tc.tile_pool(name="atn", bufs=FT))
    pt_pool = ctx.enter_context(tc.tile_pool(name="pt", bufs=35))
    nrm_pool = ctx.enter_context(tc.tile_pool(name="nrm", bufs=2))
    ob_pool = ctx.enter_context(tc.tile_pool(name="ob", bufs=2))
    wo_pool = ctx.enter_context(tc.tile_pool(name="wop", bufs=1))
    ps2 = ctx.enter_context(tc.tile_pool(name="ps2", bufs=3, space="PSUM"))
    ps1 = ctx.enter_context(tc.tile_pool(name="ps1", bufs=2, space="PSUM"))

    KT = [kt_pool.tile([P, S], BF16, tag="kt", name="kt") for _ in range(FT)]
    QT = [qt_pool.tile([P, S], BF16, tag="qt", name="qt") for _ in range(FT)]
    VA = [va_pool.tile([P, HC * DP1], BF16, tag="va", name="va")
          for _ in range(NKT)]
    ATN = [atn_pool.tile([P, S], BF16, tag="at", name="at") for _ in range(FT)]
    PT = {}

    # Streamed activations / weights (per-et tiles; first chain starts after
    # just its first two small DMAs land)
    xk_p = ctx.enter_context(tc.tile_pool(name="xk", bufs=ET))
    xq_p = ctx.enter_context(tc.tile_pool(name="xq", bufs=ET))
    wk_p = ctx.enter_context(tc.tile_pool(name="wk", bufs=FT))
    wq_p = ctx.enter_context(tc.tile_pool(name="wq", bufs=FT))
    xv_p = ctx.enter_context(tc.tile_pool(name="xv", bufs=ET))
    wv_p = ctx.enter_context(tc.tile_pool(name="wv", bufs=ET))

    xk_t = [xk_p.tile([P, S], BF16, tag="xk", name="xk") for _ in range(ET)]
    xq_t = [xq_p.tile([P, S], BF16, tag="xq", name="xq") for _ in range(ET)]
    wk_t = [wk_p.tile([P, ET * P], BF16, tag="wk", name="wk")
            for _ in range(FT)]
    wq_t = [wq_p.tile([P, ET * P], BF16, tag="wq", name="wq")
            for _ in range(FT)]
    xv_t = [xv_p.tile([P, S], BF16, tag="xv", name="xv") for _ in range(ET)]
    wv_t = [wv_p.tile([P, EH], BF16, tag="wv", name="wv") for _ in range(ET)]
    wo_t = wo_pool.tile([P, FT * E], BF16, tag="wo", name="wo")

    def _wf_load(tt, dram, ft):
        nc.sync.dma_start(
            tt[:].rearrange("p (a s) -> p a s", s=P),
            dram.ap()[:, ft * P:(ft + 1) * P].rearrange(
                "(a p) s -> p a s", p=P))

    def emit_loads():
        for et in range(ET):
            nc.sync.dma_start(xk_t[et][:], xkT.ap()[et * P:(et + 1) * P, :])
        _wf_load(wk_t[0], wk, 0)
        for et in range(ET):
            nc.sync.dma_start(xq_t[et][:], xqT.ap()[et * P:(et + 1) * P, :])
        _wf_load(wq_t[0], wq, 0)
        for ft in range(1, FT):
            _wf_load(wk_t[ft], wk, ft)
            _wf_load(wq_t[ft], wq, ft)
        for et in range(ET):
            nc.sync.dma_start(xv_t[et][:], xvT.ap()[et * P:(et + 1) * P, :])
            nc.sync.dma_start(wv_t[et][:], wv.ap()[et * P:(et + 1) * P, :])
        nc.sync.dma_start(
            wo_t[:].rearrange("p (a s) -> p a s", s=E),
            wo.ap()[:].rearrange("(a p) s -> p a s", p=P))

    emit_loads()
    for tt in range(NKT):
        ones3 = VA[tt][:].rearrange("p (h c) -> p h c", c=DP1)[:, :, D:DP1]
        nc.gpsimd.dma_start(
            ones3, ones.ap()[:, 0:HC].rearrange("p (h c) -> p h c", c=1))

    return dict(ctx=ctx, KT=KT, QT=QT, VA=VA, ATN=ATN, on_t=on_t,
                kt_b=(bq_t, bk_t, mb_t, bo_t),
                xk_t=xk_t, xq_t=xq_t, wk_t=wk_t, wq_t=wq_t,
                xv_t=xv_t, wv_t=wv_t, wo_t=wo_t,
                ps2=ps2, ps1=ps1, pt_pool=pt_pool, nrm_pool=nrm_pool,
                ob_pool=ob_pool, emit_loads=emit_loads)


def _emit_compute(nc, tc, t, h, out_key="out", refill=False,
                  emit_out=True, emit_pv3=True):
    out = t[out_key]
    KT, QT, VA, ATN = h["KT"], h["QT"], h["VA"], h["ATN"]
    bq_t, bk_t, mb_t, bo_t = h["kt_b"]
    xk_t, xq_t, wk_t, wq_t = h["xk_t"], h["xq_t"], h["wk_t"], h["wq_t"]
    xv_t, wv_t, wo_t = h["xv_t"], h["wv_t"], h["wo_t"]
    ps2, ps1 = h["ps2"], h["ps1"]
    pt_pool, nrm_pool, ob_pool = h["pt_pool"], h["nrm_pool"], h["ob_pool"]
    PT = {}

    proj_ps = {}

    def proj_half(ft, kh, w_t, x_t, dst, bias_t):
        # half-chain unit (8 mm); evict with the second half
        if kh == 0:
            proj_ps[id(dst) ^ ft] = ps2.tile([P, S], F32, tag="p2", name="p2")
        ps = proj_ps[id(dst) ^ ft]
        for et in range(ET):
            nc.tensor.matmul(
                ps[:, kh * 512:(kh + 1) * 512],
                w_t[ft][:, et * P:(et + 1) * P],
                x_t[et][:, kh * 512:(kh + 1) * 512],
                start=(et == 0), stop=(et == ET - 1))
        if kh == 1:
            nc.vector.tensor_scalar_add(dst[ft][:], ps[:],
                                        bias_t[:, ft:ft + 1])

    def vproj(tt):
        ps = ps1.tile([P, 512], F32, tag="p1", name="p1")
        for et in range(ET):
            nc.tensor.matmul(
                ps[:], xv_t[et][:, tt * P:(tt + 1) * P], wv_t[et][:],
                start=(et == 0), stop=(et == ET - 1))
        va3 = VA[tt][:].rearrange("p (h c) -> p h c", c=DP1)[:, :, 0:D]
        ps3 = ps[:].rearrange("p (h c) -> p h c", c=D)
        nc.vector.tensor_copy(va3, ps3)

    def energy(ft, kt):
        pe_h = [ps2.tile([P, S], F32, tag="p2", name="p2") for _ in range(2)]
        for qh in range(2):
            for hh in range(2):
                hp = hh * D
                nc.tensor.matmul(
                    pe_h[hh][:, qh * 512:(qh + 1) * 512],
                    KT[ft][hp:hp + D, kt * P:(kt + 1) * P],
                    QT[ft][hp:hp + D, qh * 512:(qh + 1) * 512],
                    start=True, stop=True, tile_position=(hp, 0))
        for hh in range(2):
            pt = pt_pool.tile([P, S], BF16, tag="pt", name="pt")
            nc.scalar.activation(pt[:], pe_h[hh][:], AF.Exp,
                                 bias=mb_t[:, kt:kt + 1])
            PT[(ft, hh, kt)] = pt

    pv_live = {}

    def pv_mm(ft, hh, qh):
        # PV accumulation + reciprocal; the broadcast+multiply finisher is
        # deferred (pv_fin) so its Pool/DVE latency stays off the window
        # critical path.
        hc = ft * 2 + hh
        ps = ps1.tile([DP1, 512], F32, tag="p1", name="p1")
        for kt in range(NKT):
            nc.tensor.matmul(
                ps[:], VA[kt][:, hc * DP1:(hc + 1) * DP1],
                PT[(ft, hh, kt)][:, qh * 512:(qh + 1) * 512],
                start=(kt == 0), stop=(kt == NKT - 1))
        rec = nrm_pool.tile([1, 512], F32, tag="rec", name="rec")
        nc.vector.reciprocal(rec[0:1, :], ps[D:DP1, :])
        pv_live[(ft, hh, qh)] = (ps, rec)

    def pv_fin(ft, hh, qh):
        ps, rec = pv_live.pop((ft, hh, qh))
        bc = nrm_pool.tile([D, 512], F32, tag="bc", name="bc")
        nc.gpsimd.partition_broadcast(bc[0:D, :], rec[0:1, :])
        nc.vector.tensor_mul(
            ATN[ft][hh * D:(hh + 1) * D, qh * 512:(qh + 1) * 512],
            ps[0:D, :], bc[0:D, :])

    def pv(ft, hh, qh):
        pv_mm(ft, hh, qh)
        pv_fin(ft, hh, qh)

    def outp(qt):
        # Both 512-wide output halves accumulate into one [128,1024] psum
        # tile: 8 MMs, then a single DVE bias-add and a single DMA store.
        ps = ps2.tile([P, S], F32, tag="p2", name="p2")
        for ft in range(FT):
            for eb in range(2):
                nc.tensor.matmul(
                    ps[:, eb * 512:(eb + 1) * 512],
                    ATN[ft][:, qt * P:(qt + 1) * P],
                    wo_t[:, ft * E + eb * 512:ft * E + (eb + 1) * 512],
                    start=(ft == 0), stop=(ft == FT - 1))
        ob = ob_pool.tile([P, S], F32, tag="ob", name="ob")
        nc.vector.tensor_add(ob[:], ps[:], bo_t[:])
        nc.scalar.dma_start(out.ap()[qt * P:(qt + 1) * P, :], ob[:])

    # ---- hand-woven emission order ----
    # One ~2.3us filler unit after each energy group paces E emission at the
    # ACT exp drain rate (~2.7us/group). V runs in windows 0-1 so pv(ft)
    # lands at window ft+2 (PT in flight <= 34, pool 35). The tail orders
    # pv(3) by q-half so OUT chains for qt 0-3 overlap pv(3,*,1).
    def K(ft, kh):
        return lambda: proj_half(ft, kh, wk_t, xk_t, KT, bk_t)

    def Q(ft, kh):
        return lambda: proj_half(ft, kh, wq_t, xq_t, QT, bq_t)

    def seq(*fs):
        return lambda: [f() for f in fs]

    def MM(ft, hh, qh):
        return lambda: pv_mm(ft, hh, qh)

    def FIN(ft, hh, qh):
        return lambda: pv_fin(ft, hh, qh)

    for f in (K(0, 0), K(0, 1), Q(0, 0), Q(0, 1)):
        f()
    fillers = [
        K(1, 0), K(1, 1), Q(1, 0), Q(1, 1),
    ] + [lambda tt=tt: vproj(tt) for tt in range(NKT)] + [
        K(2, 0), K(2, 1), Q(2, 0), Q(2, 1),
        MM(0, 0, 0),
        seq(FIN(0, 0, 0), MM(0, 0, 1)),
        seq(FIN(0, 0, 1), MM(0, 1, 0)),
        seq(FIN(0, 1, 0), MM(0, 1, 1)),
        seq(FIN(0, 1, 1), K(3, 0)),
        K(3, 1), Q(3, 0), Q(3, 1),
        MM(1, 0, 0),
        seq(FIN(1, 0, 0), MM(1, 0, 1)),
        seq(FIN(1, 0, 1), MM(1, 1, 0)),
        seq(FIN(1, 1, 0), MM(1, 1, 1)),
        seq(FIN(1, 1, 1), MM(2, 0, 0)),
        seq(FIN(2, 0, 0), MM(2, 0, 1)),
        seq(FIN(2, 0, 1), MM(2, 1, 0)),
        seq(FIN(2, 1, 0), MM(2, 1, 1)),
    ]
    fi = iter(fillers)
    for ft in range(FT):
        for kt in range(NKT):
            energy(ft, kt)
            f = next(fi, None)
            if f is not None:
                f()
    pv_fin(2, 1, 1)
    if emit_pv3:
        pv_mm(3, 0, 0)
        pv_mm(3, 1, 0)
        pv_fin(3, 0, 0)
        pv_mm(3, 0, 1)
        pv_fin(3, 1, 0)
    if emit_out:
        outp(0)
        if emit_pv3:
            pv_mm(3, 1, 1)
        outp(1)
        if emit_pv3:
            pv_fin(3, 0, 1)
            pv_fin(3, 1, 1)
        for qt in range(2, ET):
            outp(qt)
    elif emit_pv3:
        pv_mm(3, 1, 1)
        pv_fin(3, 0, 1)
        pv_fin(3, 1, 1)
    if refill:
        h["emit_loads"]()


def build_nc(repeats=1, hw_loop=0, refill=True, emit_out=True, emit_pv3=True):
    nc = bacc.Bacc()
    t = _declare(nc)
    with tile.TileContext(nc) as tc:
        h = _emit_prelude(nc, tc, t)
        if hw_loop:
            with tc.For_i(0, hw_loop, 1):
                _emit_compute(nc, tc, t, h, refill=refill,
                              emit_out=emit_out, emit_pv3=emit_pv3)
        else:
            for _ in range(repeats):
                _emit_compute(nc, tc, t, h, refill=False)
        h["ctx"].close()
    nc.finalize()
    return nc


_NC = None


def _get_nc():
    global _NC
    if _NC is None:
        _NC = build_nc()
    return _NC


def _prep_in_maps(value, key_in, query, mask, Wq, bq, Wk, bk, Wv, bv, Wo, bo):
    f = np.float32
    value = np.asarray(value, f)
    key_in = np.asarray(key_in, f)
    query = np.asarray(query, f)
    mask = np.asarray(mask)
    Wq = np.asarray(Wq, f); bq = np.asarray(bq, f)
    Wk = np.asarray(Wk, f); bk = np.asarray(bk, f)
    Wv = np.asarray(Wv, f); bv = np.asarray(bv, f)
    Wo = np.asarray(Wo, f); bo = np.asarray(bo, f)

    s = f(1.0 / np.sqrt(E))
    wqT = (Wq.T * s).astype(NPBF16)
    wkT = Wk.T.astype(NPBF16)
    wvT = Wv.T.astype(NPBF16)
    woT = Wo.T.astype(NPBF16)
    bo_eff = bo + Wo @ bv
    bo_full = np.ascontiguousarray(
        np.broadcast_to(bo_eff, (P, E))).astype(NPBF16)
    bo_zero = np.zeros((P, E), NPBF16)
    ones_t = np.ones((P, D), NPBF16)

    xT = {}
    for b in range(B):
        xT[b] = (
            np.ascontiguousarray(query[b].astype(NPBF16).T),
            np.ascontiguousarray(key_in[b].astype(NPBF16).T),
            np.ascontiguousarray(value[b].astype(NPBF16).T),
        )

    in_maps = []
    for c in range(N_CORES):
        b, g = c // 2, c % 2
        cols = slice(g * EH, (g + 1) * EH)
        mrow = mask[b, 0, 0, :]
        mb = np.where(mrow == 0, f(-50.0), f(0.0)).astype(f)
        xq_b, xk_b, xv_b = xT[b]
        in_maps.append({
            "xqT": xq_b, "xkT": xk_b, "xvT": xv_b,
            "wq": np.ascontiguousarray(wqT[:, cols]),
            "wk": np.ascontiguousarray(wkT[:, cols]),
            "wv": np.ascontiguousarray(wvT[:, cols]),
            "wo": np.ascontiguousarray(woT[cols, :]),
            "bq": np.ascontiguousarray((bq[cols] * s).reshape(FT, P).T),
            "bk": np.ascontiguousarray(bk[cols].reshape(FT, P).T),
            "maskb": np.ascontiguousarray(mb.reshape(NKT, P).T),
            "boeff": bo_full if g == 0 else bo_zero,
            "ones": ones_t,
        })
    return in_maps


def _assemble(results):
    out = np.empty((B, S, E), np.float32)
    for b in range(B):
        out[b] = results[2 * b]["out"] + results[2 * b + 1]["out"]
    return out


def kernel(value, key_in, query, mask, Wq, bq, Wk, bk, Wv, bv, Wo, bo):
    nc = _get_nc()
    in_maps = _prep_in_maps(value, key_in, query, mask,
                            Wq, bq, Wk, bk, Wv, bv, Wo, bo)
    r = run_bass_kernel_spmd(nc, in_maps, list(range(N_CORES)))
    return _assemble(r.results)


def kernel_traced(value, key_in, query, mask, Wq, bq, Wk, bk, Wv, bv, Wo, bo,
                  **trace_kwargs):
    """Like kernel() but returns (output, BassKernelResults) with profiling."""
    nc = _get_nc()
    in_maps = _prep_in_maps(value, key_in, query, mask,
                            Wq, bq, Wk, bk, Wv, bv, Wo, bo)
    r = run_bass_kernel_spmd(nc, in_maps, list(range(N_CORES)), trace=True,
                             **trace_kwargs)
    return _assemble(r.results), r



# revision 21
# speedup vs baseline: 1.0150x; 1.0139x over previous
"""Multi-head attention Trainium2 kernel (nn_MultiHeadAttention_7035156430929).

B=4, S=1024, E=1024, H=16, D=64. Sharding: 8 cores = 4 batches x 2
head-groups (tensor parallel over heads, per the hint). Each core computes 8
heads for all 1024 queries/keys of its batch: Wq/Wk/Wv column-sliced (512
features), Wo row-sliced, giving a PARTIAL output [1024, 1024] per core; the
two partials of a batch are summed on the host (the "all-reduce after fc_out"
done host-side since core outputs are gathered anyway).

Everything on device is bf16 except PSUM accumulation, softmax denominators,
biases, and the output partials (fp32). Measured-HW design points:
  - matmul N=512 ~285 ns; K=64 pairs on PE row-groups (tile_position 0/64)
    run concurrently (~308 ns/pair) -> energy QK^T paired across head pairs.
  - exp on ACT: [128,1024] = 1324 ns; energy PSUM tiles are [128,1024]
    (2 banks) so each (head, kt) needs ONE activation.
  - PE emission is hand-woven: energy groups are spaced between projection /
    PV / output chains so the ACT engine (85 us of exp) hides under the PE
    stream (~128 us) and energy PSUM slots (3x2 banks) never block.
  - In the For_i timing loop (whose per-iteration all-engine barrier
    serializes bodies), input tiles persist and each iteration RE-LOADS them
    at its tail, so the HBM traffic stays per-iteration but the next body's
    projections start right after the barrier (no DMA lead-in).
1/sqrt(E) folds into Wq/bq; bv folds into the g=0 core's output bias
(softmax rows sum to 1); key-padding mask becomes an additive per-key bias
(-50) inside the exp activation.
"""
import sys

sys.path.insert(0, "/opt/trn_rl_repo")

from contextlib import ExitStack

import numpy as np
import ml_dtypes

import concourse.bacc as bacc
import concourse.tile as tile
from concourse import mybir
from concourse.bass_utils import run_bass_kernel_spmd

B, S, E, H, D = 4, 1024, 1024, 16, 64
P = 128
N_CORES = 8
HC = 8             # heads per core
EH = 512           # features per core
FT = 4             # feature tiles of 128 (2 heads each)
ET = 8             # contraction tiles over E
NKT = 8            # key-token tiles
DP1 = D + 1        # V columns per head incl. ones column
F32 = mybir.dt.float32
BF16 = mybir.dt.bfloat16
AF = mybir.ActivationFunctionType
NPBF16 = ml_dtypes.bfloat16


def _declare(nc):
    dp = nc.declare_dram_parameter
    t = {}
    t["xqT"] = dp("xqT", [E, S], BF16, isOutput=False)   # query[b].T
    t["xkT"] = dp("xkT", [E, S], BF16, isOutput=False)
    t["xvT"] = dp("xvT", [E, S], BF16, isOutput=False)
    t["wq"] = dp("wq", [E, EH], BF16, isOutput=False)    # Wq.T col-slice, *s
    t["wk"] = dp("wk", [E, EH], BF16, isOutput=False)
    t["wv"] = dp("wv", [E, EH], BF16, isOutput=False)
    t["wo"] = dp("wo", [EH, E], BF16, isOutput=False)    # Wo.T row-slice
    t["bq"] = dp("bq", [P, FT], F32, isOutput=False)
    t["bk"] = dp("bk", [P, FT], F32, isOutput=False)
    t["maskb"] = dp("maskb", [P, NKT], F32, isOutput=False)  # 0 or -50
    t["boeff"] = dp("boeff", [P, E], BF16, isOutput=False)   # row-replicated
    t["ones"] = dp("ones", [P, D], BF16, isOutput=False)
    t["out"] = dp("out", [S, E], F32, isOutput=True)         # partial
    t["out2"] = dp("out2", [S, E], F32, isOutput=True)       # timing-loop pair
    return t


def _emit_prelude(nc, tc, t):
    """Create all pools/tiles and emit the initial input loads. Returns a
    state dict shared by every _emit_compute call (tiles persist across
    For_i iterations; each iteration refills them at its tail so the next
    iteration's projections start immediately after the loop barrier)."""
    xqT, xkT, xvT = t["xqT"], t["xkT"], t["xvT"]
    wq, wk, wv, wo = t["wq"], t["wk"], t["wv"], t["wo"]
    bq, bk, maskb, boeff = t["bq"], t["bk"], t["maskb"], t["boeff"]
    ones = t["ones"]

    ctx = ExitStack()
    const = ctx.enter_context(tc.tile_pool(name="const", bufs=1))
    bq_t = const.tile([P, FT], F32, tag="bq", name="bq")
    bk_t = const.tile([P, FT], F32, tag="bk", name="bk")
    mb_t = const.tile([P, NKT], F32, tag="mb", name="mb")
    bo_t = const.tile([P, E], BF16, tag="bo", name="bo")
    on_t = const.tile([1, D], BF16, tag="on", name="on")
    nc.gpsimd.dma_start(bq_t[:], bq.ap()[:])
    nc.gpsimd.dma_start(bk_t[:], bk.ap()[:])
    nc.gpsimd.dma_start(mb_t[:], maskb.ap()[:])
    nc.gpsimd.dma_start(bo_t[:], boeff.ap()[:])
    nc.gpsimd.dma_start(on_t[:], ones.ap()[0:1, :])

    # Persistent per-iteration intermediates
    kt_pool = ctx.enter_context(tc.tile_pool(name="ktp", bufs=FT))
    qt_pool = ctx.enter_context(tc.tile_pool(name="qtp", bufs=FT))
    va_pool = ctx.enter_context(tc.tile_pool(name="vap", bufs=NKT))
    atn_pool = ctx.enter_context(tc.tile_pool(name="atn", bufs=FT))
    pt_pool = ctx.enter_context(tc.tile_pool(name="pt", bufs=35))
    nrm_pool = ctx.enter_context(tc.tile_pool(name="nrm", bufs=2))
    ob_pool = ctx.enter_context(tc.tile_pool(name="ob", bufs=2))
    wo_pool = ctx.enter_context(tc.tile_pool(name="wop", bufs=1))
    ps2 = ctx.enter_context(tc.tile_pool(name="ps2", bufs=3, space="PSUM"))
    ps1 = ctx.enter_context(tc.tile_pool(name="ps1", bufs=2, space="PSUM"))

    KT = [kt_pool.tile([P, S], BF16, tag="kt", name="kt") for _ in range(FT)]
    QT = [qt_pool.tile([P, S], BF16, tag="qt", name="qt") for _ in range(FT)]
    VA = [va_pool.tile([P, HC * DP1], BF16, tag="va", name="va")
          for _ in range(NKT)]
    ATN = [atn_pool.tile([P, S], BF16, tag="at", name="at") for _ in range(FT)]
    PT = {}

    # Streamed activations / weights (per-et tiles; first chain starts after
    # just its first two small DMAs land)
    xk_p = ctx.enter_context(tc.tile_pool(name="xk", bufs=ET))
    xq_p = ctx.enter_context(tc.tile_pool(name="xq", bufs=ET))
    wk_p = ctx.enter_context(tc.tile_pool(name="wk", bufs=FT))
    wq_p = ctx.enter_context(tc.tile_pool(name="wq", bufs=FT))
    xv_p = ctx.enter_context(tc.tile_pool(name="xv", bufs=ET))
    wv_p = ctx.enter_context(tc.tile_pool(name="wv", bufs=ET))

    xk_t = [xk_p.tile([P, S], BF16, tag="xk", name="xk") for _ in range(ET)]
    xq_t = [xq_p.tile([P, S], BF16, tag="xq", name="xq") for _ in range(ET)]
    wk_t = [wk_p.tile([P, ET * P], BF16, tag="wk", name="wk")
            for _ in range(FT)]
    wq_t = [wq_p.tile([P, ET * P], BF16, tag="wq", name="wq")
            for _ in range(FT)]
    xv_t = [xv_p.tile([P, S], BF16, tag="xv", name="xv") for _ in range(ET)]
    wv_t = [wv_p.tile([P, EH], BF16, tag="wv", name="wv") for _ in range(ET)]
    wo_t = wo_pool.tile([P, FT * E], BF16, tag="wo", name="wo")

    def _wf_load(tt, dram, ft):
        nc.sync.dma_start(
            tt[:].rearrange("p (a s) -> p a s", s=P),
            dram.ap()[:, ft * P:(ft + 1) * P].rearrange(
                "(a p) s -> p a s", p=P))

    def emit_loads():
        for et in range(ET):
            nc.sync.dma_start(xk_t[et][:], xkT.ap()[et * P:(et + 1) * P, :])
        _wf_load(wk_t[0], wk, 0)
        for et in range(ET):
            nc.sync.dma_start(xq_t[et][:], xqT.ap()[et * P:(et + 1) * P, :])
        _wf_load(wq_t[0], wq, 0)
        for ft in range(1, FT):
            _wf_load(wk_t[ft], wk, ft)
            _wf_load(wq_t[ft], wq, ft)
        for et in range(ET):
            nc.sync.dma_start(xv_t[et][:], xvT.ap()[et * P:(et + 1) * P, :])
            nc.sync.dma_start(wv_t[et][:], wv.ap()[et * P:(et + 1) * P, :])
        nc.sync.dma_start(
            wo_t[:].rearrange("p (a s) -> p a s", s=E),
            wo.ap()[:].rearrange("(a p) s -> p a s", p=P))

    emit_loads()
    for tt in range(NKT):
        ones3 = VA[tt][:].rearrange("p (h c) -> p h c", c=DP1)[:, :, D:DP1]
        nc.gpsimd.dma_start(
            ones3, ones.ap()[:, 0:HC].rearrange("p (h c) -> p h c", c=1))

    return dict(ctx=ctx, KT=KT, QT=QT, VA=VA, ATN=ATN, on_t=on_t,
                kt_b=(bq_t, bk_t, mb_t, bo_t),
                xk_t=xk_t, xq_t=xq_t, wk_t=wk_t, wq_t=wq_t,
                xv_t=xv_t, wv_t=wv_t, wo_t=wo_t,
                ps2=ps2, ps1=ps1, pt_pool=pt_pool, nrm_pool=nrm_pool,
                ob_pool=ob_pool, emit_loads=emit_loads)


def _emit_compute(nc, tc, t, h, out_key="out", refill=False,
                  emit_out=True, emit_pv3=True):
    out = t[out_key]
    KT, QT, VA, ATN = h["KT"], h["QT"], h["VA"], h["ATN"]
    bq_t, bk_t, mb_t, bo_t = h["kt_b"]
    xk_t, xq_t, wk_t, wq_t = h["xk_t"], h["xq_t"], h["wk_t"], h["wq_t"]
    xv_t, wv_t, wo_t = h["xv_t"], h["wv_t"], h["wo_t"]
    ps2, ps1 = h["ps2"], h["ps1"]
    pt_pool, nrm_pool, ob_pool = h["pt_pool"], h["nrm_pool"], h["ob_pool"]
    PT = {}

    proj_ps = {}

    def proj_half(ft, kh, w_t, x_t, dst, bias_t):
        # half-chain unit (8 mm); evict with the second half
        if kh == 0:
            proj_ps[id(dst) ^ ft] = ps2.tile([P, S], F32, tag="p2", name="p2")
        ps = proj_ps[id(dst) ^ ft]
        for et in range(ET):
            nc.tensor.matmul(
                ps[:, kh * 512:(kh + 1) * 512],
                w_t[ft][:, et * P:(et + 1) * P],
                x_t[et][:, kh * 512:(kh + 1) * 512],
                start=(et == 0), stop=(et == ET - 1))
        if kh == 1:
            nc.vector.tensor_scalar_add(dst[ft][:], ps[:],
                                        bias_t[:, ft:ft + 1])

    def vproj(tt):
        ps = ps1.tile([P, 512], F32, tag="p1", name="p1")
        for et in range(ET):
            nc.tensor.matmul(
                ps[:], xv_t[et][:, tt * P:(tt + 1) * P], wv_t[et][:],
                start=(et == 0), stop=(et == ET - 1))
        va3 = VA[tt][:].rearrange("p (h c) -> p h c", c=DP1)[:, :, 0:D]
        ps3 = ps[:].rearrange("p (h c) -> p h c", c=D)
        nc.vector.tensor_copy(va3, ps3)

    def energy(ft, kt):
        pe_h = [ps2.tile([P, S], F32, tag="p2", name="p2") for _ in range(2)]
        for qh in range(2):
            for hh in range(2):
                hp = hh * D
                nc.tensor.matmul(
                    pe_h[hh][:, qh * 512:(qh + 1) * 512],
                    KT[ft][hp:hp + D, kt * P:(kt + 1) * P],
                    QT[ft][hp:hp + D, qh * 512:(qh + 1) * 512],
                    start=True, stop=True, tile_position=(hp, 0))
        for hh in range(2):
            pt = pt_pool.tile([P, S], BF16, tag="pt", name="pt")
            nc.scalar.activation(pt[:], pe_h[hh][:], AF.Exp,
                                 bias=mb_t[:, kt:kt + 1])
            PT[(ft, hh, kt)] = pt

    pv_live = {}

    def pv_mm(ft, hh, qh):
        # PV accumulation + reciprocal; the broadcast+multiply finisher is
        # deferred (pv_fin) so its Pool/DVE latency stays off the window
        # critical path.
        hc = ft * 2 + hh
        ps = ps1.tile([DP1, 512], F32, tag="p1", name="p1")
        for kt in range(NKT):
            nc.tensor.matmul(
                ps[:], VA[kt][:, hc * DP1:(hc + 1) * DP1],
                PT[(ft, hh, kt)][:, qh * 512:(qh + 1) * 512],
                start=(kt == 0), stop=(kt == NKT - 1))
        rec = nrm_pool.tile([1, 512], F32, tag="rec", name="rec")
        nc.vector.reciprocal(rec[0:1, :], ps[D:DP1, :])
        pv_live[(ft, hh, qh)] = (ps, rec)

    def pv_fin(ft, hh, qh):
        ps, rec = pv_live.pop((ft, hh, qh))
        bc = nrm_pool.tile([D, 512], F32, tag="bc", name="bc")
        nc.gpsimd.partition_broadcast(bc[0:D, :], rec[0:1, :])
        nc.vector.tensor_mul(
            ATN[ft][hh * D:(hh + 1) * D, qh * 512:(qh + 1) * 512],
            ps[0:D, :], bc[0:D, :])

    def pv(ft, hh, qh):
        pv_mm(ft, hh, qh)
        pv_fin(ft, hh, qh)

    def outp(qt):
        # Both 512-wide output halves accumulate into one [128,1024] psum
        # tile: 8 MMs, then a single DVE bias-add and a single DMA store.
        ps = ps2.tile([P, S], F32, tag="p2", name="p2")
        for ft in range(FT):
            for eb in range(2):
                nc.tensor.matmul(
                    ps[:, eb * 512:(eb + 1) * 512],
                    ATN[ft][:, qt * P:(qt + 1) * P],
                    wo_t[:, ft * E + eb * 512:ft * E + (eb + 1) * 512],
                    start=(ft == 0), stop=(ft == FT - 1))
        ob = ob_pool.tile([P, S], F32, tag="ob", name="ob")
        nc.vector.tensor_add(ob[:], ps[:], bo_t[:])
        nc.scalar.dma_start(out.ap()[qt * P:(qt + 1) * P, :], ob[:])

    # ---- hand-woven emission order ----
    # One ~2.3us filler unit after each energy group paces E emission at the
    # ACT exp drain rate (~2.7us/group). V runs in windows 0-1 so pv(ft)
    # lands at window ft+2 (PT in flight <= 34, pool 35). The tail orders
    # pv(3) by q-half so OUT chains for qt 0-3 overlap pv(3,*,1).
    def K(ft, kh):
        return lambda: proj_half(ft, kh, wk_t, xk_t, KT, bk_t)

    def Q(ft, kh):
        return lambda: proj_half(ft, kh, wq_t, xq_t, QT, bq_t)

    def seq(*fs):
        return lambda: [f() for f in fs]

    def MM(ft, hh, qh):
        return lambda: pv_mm(ft, hh, qh)

    def FIN(ft, hh, qh):
        return lambda: pv_fin(ft, hh, qh)

    for f in (K(0, 0), K(0, 1), Q(0, 0), Q(0, 1)):
        f()
    fillers = [
        K(1, 0), K(1, 1), Q(1, 0), Q(1, 1),
    ] + [lambda tt=tt: vproj(tt) for tt in range(NKT)] + [
        K(2, 0), K(2, 1), Q(2, 0), Q(2, 1),
        lambda: pv(0, 0, 0), lambda: pv(0, 0, 1),
        lambda: pv(0, 1, 0), lambda: pv(0, 1, 1),
        K(3, 0), K(3, 1), Q(3, 0), Q(3, 1),
        lambda: pv(1, 0, 0), lambda: pv(1, 0, 1),
        lambda: pv(1, 1, 0), lambda: pv(1, 1, 1),
        lambda: pv(2, 0, 0), lambda: pv(2, 0, 1),
        lambda: pv(2, 1, 0), lambda: pv(2, 1, 1),
    ]
    fi = iter(fillers)
    for ft in range(FT):
        for kt in range(NKT):
            energy(ft, kt)
            f = next(fi, None)
            if f is not None:
                f()
    if emit_pv3:
        pv(3, 0, 0)
        pv(3, 1, 0)
    if emit_out:
        for qt in range(4):
            outp(qt)
            if qt < 2 and emit_pv3:
                pv(3, qt, 1)
        for qt in range(4, ET):
            outp(qt)
    elif emit_pv3:
        pv(3, 0, 1)
        pv(3, 1, 1)
    if refill:
        h["emit_loads"]()


def build_nc(repeats=1, hw_loop=0, refill=True, emit_out=True, emit_pv3=True):
    nc = bacc.Bacc()
    t = _declare(nc)
    with tile.TileContext(nc) as tc:
        h = _emit_prelude(nc, tc, t)
        if hw_loop:
            with tc.For_i(0, hw_loop, 1):
                _emit_compute(nc, tc, t, h, refill=refill,
                              emit_out=emit_out, emit_pv3=emit_pv3)
        else:
            for _ in range(repeats):
                _emit_compute(nc, tc, t, h, refill=False)
        h["ctx"].close()
    nc.finalize()
    return nc


_NC = None


def _get_nc():
    global _NC
    if _NC is None:
        _NC = build_nc()
    return _NC


def _prep_in_maps(value, key_in, query, mask, Wq, bq, Wk, bk, Wv, bv, Wo, bo):
    f = np.float32
    value = np.asarray(value, f)
    key_in = np.asarray(key_in, f)
    query = np.asarray(query, f)
    mask = np.asarray(mask)
    Wq = np.asarray(Wq, f); bq = np.asarray(bq, f)
    Wk = np.asarray(Wk, f); bk = np.asarray(bk, f)
    Wv = np.asarray(Wv, f); bv = np.asarray(bv, f)
    Wo = np.asarray(Wo, f); bo = np.asarray(bo, f)

    s = f(1.0 / np.sqrt(E))
    wqT = (Wq.T * s).astype(NPBF16)
    wkT = Wk.T.astype(NPBF16)
    wvT = Wv.T.astype(NPBF16)
    woT = Wo.T.astype(NPBF16)
    bo_eff = bo + Wo @ bv
    bo_full = np.ascontiguousarray(
        np.broadcast_to(bo_eff, (P, E))).astype(NPBF16)
    bo_zero = np.zeros((P, E), NPBF16)
    ones_t = np.ones((P, D), NPBF16)

    xT = {}
    for b in range(B):
        xT[b] = (
            np.ascontiguousarray(query[b].astype(NPBF16).T),
            np.ascontiguousarray(key_in[b].astype(NPBF16).T),
            np.ascontiguousarray(value[b].astype(NPBF16).T),
        )

    in_maps = []
    for c in range(N_CORES):
        b, g = c // 2, c % 2
        cols = slice(g * EH, (g + 1) * EH)
        mrow = mask[b, 0, 0, :]
        mb = np.where(mrow == 0, f(-50.0), f(0.0)).astype(f)
        xq_b, xk_b, xv_b = xT[b]
        in_maps.append({
            "xqT": xq_b, "xkT": xk_b, "xvT": xv_b,
            "wq": np.ascontiguousarray(wqT[:, cols]),
            "wk": np.ascontiguousarray(wkT[:, cols]),
            "wv": np.ascontiguousarray(wvT[:, cols]),
            "wo": np.ascontiguousarray(woT[cols, :]),
            "bq": np.ascontiguousarray((bq[cols] * s).reshape(FT, P).T),
            "bk": np.ascontiguousarray(bk[cols].reshape(FT, P).T),
            "maskb": np.ascontiguousarray(mb.reshape(NKT, P).T),
            "boeff": bo_full if g == 0 else bo_zero,
            "ones": ones_t,
        })
    return in_maps


def _assemble(results):
    out = np.empty((B, S, E), np.float32)
    for b in range(B):
        out[b] = results[2 * b]["out"] + results[2 * b + 1]["out"]
    return out


def kernel(value, key_in, query, mask, Wq, bq, Wk, bk, Wv, bv, Wo, bo):
    nc = _get_nc()
    in_maps = _prep_in_maps(value, key_in, query, mask,
                            Wq, bq, Wk, bk, Wv, bv, Wo, bo)
    r = run_bass_kernel_spmd(nc, in_maps, list(range(N_CORES)))
    return _assemble(r.results)


def kernel_traced(value, key_in, query, mask, Wq, bq, Wk, bk, Wv, bv, Wo, bo,
                  **trace_kwargs):
    """Like kernel() but returns (output, BassKernelResults) with profiling."""
    nc = _get_nc()
    in_maps = _prep_in_maps(value, key_in, query, mask,
                            Wq, bq, Wk, bk, Wv, bv, Wo, bo)
    r = run_bass_kernel_spmd(nc, in_maps, list(range(N_CORES)), trace=True,
                             **trace_kwargs)
    return _assemble(r.results), r



# revision 22
# speedup vs baseline: 1.0275x; 1.0123x over previous
"""Multi-head attention Trainium2 kernel (nn_MultiHeadAttention_7035156430929).

B=4, S=1024, E=1024, H=16, D=64. Sharding: 8 cores = 4 batches x 2
head-groups (tensor parallel over heads, per the hint). Each core computes 8
heads for all 1024 queries/keys of its batch: Wq/Wk/Wv column-sliced (512
features), Wo row-sliced, giving a PARTIAL output [1024, 1024] per core; the
two partials of a batch are summed on the host (the "all-reduce after fc_out"
done host-side since core outputs are gathered anyway).

Everything on device is bf16 except PSUM accumulation, softmax denominators,
biases, and the output partials (fp32). Measured-HW design points:
  - matmul N=512 ~285 ns; K=64 pairs on PE row-groups (tile_position 0/64)
    run concurrently (~308 ns/pair) -> energy QK^T paired across head pairs.
  - exp on ACT: [128,1024] = 1324 ns; energy PSUM tiles are [128,1024]
    (2 banks) so each (head, kt) needs ONE activation.
  - PE emission is hand-woven: energy groups are spaced between projection /
    PV / output chains so the ACT engine (85 us of exp) hides under the PE
    stream (~128 us) and energy PSUM slots (3x2 banks) never block.
  - In the For_i timing loop (whose per-iteration all-engine barrier
    serializes bodies), input tiles persist and each iteration RE-LOADS them
    at its tail, so the HBM traffic stays per-iteration but the next body's
    projections start right after the barrier (no DMA lead-in).
1/sqrt(E) folds into Wq/bq; bv folds into the g=0 core's output bias
(softmax rows sum to 1); key-padding mask becomes an additive per-key bias
(-50) inside the exp activation.
"""
import sys

sys.path.insert(0, "/opt/trn_rl_repo")

from contextlib import ExitStack

import numpy as np
import ml_dtypes

import concourse.bacc as bacc
import concourse.tile as tile
from concourse import mybir
from concourse.bass_utils import run_bass_kernel_spmd

B, S, E, H, D = 4, 1024, 1024, 16, 64
P = 128
N_CORES = 8
HC = 8             # heads per core
EH = 512           # features per core
FT = 4             # feature tiles of 128 (2 heads each)
ET = 8             # contraction tiles over E
NKT = 8            # key-token tiles
DP1 = D + 1        # V columns per head incl. ones column
F32 = mybir.dt.float32
BF16 = mybir.dt.bfloat16
AF = mybir.ActivationFunctionType
NPBF16 = ml_dtypes.bfloat16


def _declare(nc):
    dp = nc.declare_dram_parameter
    t = {}
    t["xqT"] = dp("xqT", [E, S], BF16, isOutput=False)   # query[b].T
    t["xkT"] = dp("xkT", [E, S], BF16, isOutput=False)
    t["xvT"] = dp("xvT", [E, S], BF16, isOutput=False)
    t["wq"] = dp("wq", [E, EH], BF16, isOutput=False)    # Wq.T col-slice, *s
    t["wk"] = dp("wk", [E, EH], BF16, isOutput=False)
    t["wv"] = dp("wv", [E, EH], BF16, isOutput=False)
    t["wo"] = dp("wo", [EH, E], BF16, isOutput=False)    # Wo.T row-slice
    t["bq"] = dp("bq", [P, FT], F32, isOutput=False)
    t["bk"] = dp("bk", [P, FT], F32, isOutput=False)
    t["maskb"] = dp("maskb", [P, NKT], F32, isOutput=False)  # 0 or -50
    t["boeff"] = dp("boeff", [P, E], BF16, isOutput=False)   # row-replicated
    t["ones"] = dp("ones", [P, D], BF16, isOutput=False)
    t["out"] = dp("out", [S, E], F32, isOutput=True)         # partial
    t["out2"] = dp("out2", [S, E], F32, isOutput=True)       # timing-loop pair
    return t


def _emit_prelude(nc, tc, t):
    """Create all pools/tiles and emit the initial input loads. Returns a
    state dict shared by every _emit_compute call (tiles persist across
    For_i iterations; each iteration refills them at its tail so the next
    iteration's projections start immediately after the loop barrier)."""
    xqT, xkT, xvT = t["xqT"], t["xkT"], t["xvT"]
    wq, wk, wv, wo = t["wq"], t["wk"], t["wv"], t["wo"]
    bq, bk, maskb, boeff = t["bq"], t["bk"], t["maskb"], t["boeff"]
    ones = t["ones"]

    ctx = ExitStack()
    const = ctx.enter_context(tc.tile_pool(name="const", bufs=1))
    bq_t = const.tile([P, FT], F32, tag="bq", name="bq")
    bk_t = const.tile([P, FT], F32, tag="bk", name="bk")
    mb_t = const.tile([P, NKT], F32, tag="mb", name="mb")
    bo_t = const.tile([P, E], BF16, tag="bo", name="bo")
    on_t = const.tile([1, D], BF16, tag="on", name="on")
    nc.gpsimd.dma_start(bq_t[:], bq.ap()[:])
    nc.gpsimd.dma_start(bk_t[:], bk.ap()[:])
    nc.gpsimd.dma_start(mb_t[:], maskb.ap()[:])
    nc.gpsimd.dma_start(bo_t[:], boeff.ap()[:])
    nc.gpsimd.dma_start(on_t[:], ones.ap()[0:1, :])

    # Persistent per-iteration intermediates
    kt_pool = ctx.enter_context(tc.tile_pool(name="ktp", bufs=FT))
    qt_pool = ctx.enter_context(tc.tile_pool(name="qtp", bufs=FT))
    va_pool = ctx.enter_context(tc.tile_pool(name="vap", bufs=NKT))
    atn_pool = ctx.enter_context(tc.tile_pool(name="atn", bufs=FT))
    pt_pool = ctx.enter_context(tc.tile_pool(name="pt", bufs=35))
    nrm_pool = ctx.enter_context(tc.tile_pool(name="nrm", bufs=2))
    ob_pool = ctx.enter_context(tc.tile_pool(name="ob", bufs=2))
    wo_pool = ctx.enter_context(tc.tile_pool(name="wop", bufs=1))
    ps2 = ctx.enter_context(tc.tile_pool(name="ps2", bufs=3, space="PSUM"))
    ps1 = ctx.enter_context(tc.tile_pool(name="ps1", bufs=2, space="PSUM"))

    KT = [kt_pool.tile([P, S], BF16, tag="kt", name="kt") for _ in range(FT)]
    QT = [qt_pool.tile([P, S], BF16, tag="qt", name="qt") for _ in range(FT)]
    VA = [va_pool.tile([P, HC * DP1], BF16, tag="va", name="va")
          for _ in range(NKT)]
    ATN = [atn_pool.tile([P, S], BF16, tag="at", name="at") for _ in range(FT)]
    PT = {}

    # Streamed activations / weights (per-et tiles; first chain starts after
    # just its first two small DMAs land)
    xk_p = ctx.enter_context(tc.tile_pool(name="xk", bufs=ET))
    xq_p = ctx.enter_context(tc.tile_pool(name="xq", bufs=ET))
    wk_p = ctx.enter_context(tc.tile_pool(name="wk", bufs=FT))
    wq_p = ctx.enter_context(tc.tile_pool(name="wq", bufs=FT))
    xv_p = ctx.enter_context(tc.tile_pool(name="xv", bufs=ET))
    wv_p = ctx.enter_context(tc.tile_pool(name="wv", bufs=ET))

    xk_t = [xk_p.tile([P, S], BF16, tag="xk", name="xk") for _ in range(ET)]
    xq_t = [xq_p.tile([P, S], BF16, tag="xq", name="xq") for _ in range(ET)]
    wk_t = [wk_p.tile([P, ET * P], BF16, tag="wk", name="wk")
            for _ in range(FT)]
    wq_t = [wq_p.tile([P, ET * P], BF16, tag="wq", name="wq")
            for _ in range(FT)]
    xv_t = [xv_p.tile([P, S], BF16, tag="xv", name="xv") for _ in range(ET)]
    wv_t = [wv_p.tile([P, EH], BF16, tag="wv", name="wv") for _ in range(ET)]
    wo_t = wo_pool.tile([P, FT * E], BF16, tag="wo", name="wo")

    def _wf_load(tt, dram, ft):
        nc.sync.dma_start(
            tt[:].rearrange("p (a s) -> p a s", s=P),
            dram.ap()[:, ft * P:(ft + 1) * P].rearrange(
                "(a p) s -> p a s", p=P))

    def emit_loads():
        for et in range(ET):
            nc.sync.dma_start(xk_t[et][:], xkT.ap()[et * P:(et + 1) * P, :])
        _wf_load(wk_t[0], wk, 0)
        for et in range(ET):
            nc.sync.dma_start(xq_t[et][:], xqT.ap()[et * P:(et + 1) * P, :])
        _wf_load(wq_t[0], wq, 0)
        for ft in range(1, FT):
            _wf_load(wk_t[ft], wk, ft)
            _wf_load(wq_t[ft], wq, ft)
        for et in range(ET):
            nc.sync.dma_start(xv_t[et][:], xvT.ap()[et * P:(et + 1) * P, :])
            nc.sync.dma_start(wv_t[et][:], wv.ap()[et * P:(et + 1) * P, :])
        nc.sync.dma_start(
            wo_t[:].rearrange("p (a s) -> p a s", s=E),
            wo.ap()[:].rearrange("(a p) s -> p a s", p=P))

    emit_loads()
    for tt in range(NKT):
        ones3 = VA[tt][:].rearrange("p (h c) -> p h c", c=DP1)[:, :, D:DP1]
        nc.gpsimd.dma_start(
            ones3, ones.ap()[:, 0:HC].rearrange("p (h c) -> p h c", c=1))

    return dict(ctx=ctx, KT=KT, QT=QT, VA=VA, ATN=ATN, on_t=on_t,
                kt_b=(bq_t, bk_t, mb_t, bo_t),
                xk_t=xk_t, xq_t=xq_t, wk_t=wk_t, wq_t=wq_t,
                xv_t=xv_t, wv_t=wv_t, wo_t=wo_t,
                ps2=ps2, ps1=ps1, pt_pool=pt_pool, nrm_pool=nrm_pool,
                ob_pool=ob_pool, emit_loads=emit_loads)


def _emit_compute(nc, tc, t, h, out_key="out", refill=False,
                  emit_out=True, emit_pv3=True):
    out = t[out_key]
    KT, QT, VA, ATN = h["KT"], h["QT"], h["VA"], h["ATN"]
    bq_t, bk_t, mb_t, bo_t = h["kt_b"]
    xk_t, xq_t, wk_t, wq_t = h["xk_t"], h["xq_t"], h["wk_t"], h["wq_t"]
    xv_t, wv_t, wo_t = h["xv_t"], h["wv_t"], h["wo_t"]
    ps2, ps1 = h["ps2"], h["ps1"]
    pt_pool, nrm_pool, ob_pool = h["pt_pool"], h["nrm_pool"], h["ob_pool"]
    PT = {}

    proj_ps = {}

    def proj_half(ft, kh, w_t, x_t, dst, bias_t):
        # half-chain unit (8 mm); evict with the second half
        if kh == 0:
            proj_ps[id(dst) ^ ft] = ps2.tile([P, S], F32, tag="p2", name="p2")
        ps = proj_ps[id(dst) ^ ft]
        for et in range(ET):
            nc.tensor.matmul(
                ps[:, kh * 512:(kh + 1) * 512],
                w_t[ft][:, et * P:(et + 1) * P],
                x_t[et][:, kh * 512:(kh + 1) * 512],
                start=(et == 0), stop=(et == ET - 1))
        if kh == 1:
            nc.vector.tensor_scalar_add(dst[ft][:], ps[:],
                                        bias_t[:, ft:ft + 1])

    def vproj(tt):
        ps = ps1.tile([P, 512], F32, tag="p1", name="p1")
        for et in range(ET):
            nc.tensor.matmul(
                ps[:], xv_t[et][:, tt * P:(tt + 1) * P], wv_t[et][:],
                start=(et == 0), stop=(et == ET - 1))
        va3 = VA[tt][:].rearrange("p (h c) -> p h c", c=DP1)[:, :, 0:D]
        ps3 = ps[:].rearrange("p (h c) -> p h c", c=D)
        nc.vector.tensor_copy(va3, ps3)

    def energy(ft, kt):
        pe_h = [ps2.tile([P, S], F32, tag="p2", name="p2") for _ in range(2)]
        for qh in range(2):
            for hh in range(2):
                hp = hh * D
                nc.tensor.matmul(
                    pe_h[hh][:, qh * 512:(qh + 1) * 512],
                    KT[ft][hp:hp + D, kt * P:(kt + 1) * P],
                    QT[ft][hp:hp + D, qh * 512:(qh + 1) * 512],
                    start=True, stop=True, tile_position=(hp, 0))
        for hh in range(2):
            pt = pt_pool.tile([P, S], BF16, tag="pt", name="pt")
            nc.scalar.activation(pt[:], pe_h[hh][:], AF.Exp,
                                 bias=mb_t[:, kt:kt + 1])
            PT[(ft, hh, kt)] = pt

    pv_live = {}

    def pv_mm(ft, hh, qh):
        # PV accumulation + reciprocal; the broadcast+multiply finisher is
        # deferred (pv_fin) so its Pool/DVE latency stays off the window
        # critical path.
        hc = ft * 2 + hh
        ps = ps1.tile([DP1, 512], F32, tag="p1", name="p1")
        for kt in range(NKT):
            nc.tensor.matmul(
                ps[:], VA[kt][:, hc * DP1:(hc + 1) * DP1],
                PT[(ft, hh, kt)][:, qh * 512:(qh + 1) * 512],
                start=(kt == 0), stop=(kt == NKT - 1))
        rec = nrm_pool.tile([1, 512], F32, tag="rec", name="rec")
        nc.vector.reciprocal(rec[0:1, :], ps[D:DP1, :])
        pv_live[(ft, hh, qh)] = (ps, rec)

    def pv_fin(ft, hh, qh):
        ps, rec = pv_live.pop((ft, hh, qh))
        bc = nrm_pool.tile([D, 512], F32, tag="bc", name="bc")
        nc.gpsimd.partition_broadcast(bc[0:D, :], rec[0:1, :])
        nc.vector.tensor_mul(
            ATN[ft][hh * D:(hh + 1) * D, qh * 512:(qh + 1) * 512],
            ps[0:D, :], bc[0:D, :])

    def pv(ft, hh, qh):
        pv_mm(ft, hh, qh)
        pv_fin(ft, hh, qh)

    def outp(qt):
        # Both 512-wide output halves accumulate into one [128,1024] psum
        # tile: 8 MMs, then a single DVE bias-add and a single DMA store.
        ps = ps2.tile([P, S], F32, tag="p2", name="p2")
        for ft in range(FT):
            for eb in range(2):
                nc.tensor.matmul(
                    ps[:, eb * 512:(eb + 1) * 512],
                    ATN[ft][:, qt * P:(qt + 1) * P],
                    wo_t[:, ft * E + eb * 512:ft * E + (eb + 1) * 512],
                    start=(ft == 0), stop=(ft == FT - 1))
        ob = ob_pool.tile([P, S], F32, tag="ob", name="ob")
        nc.vector.tensor_add(ob[:], ps[:], bo_t[:])
        nc.scalar.dma_start(out.ap()[qt * P:(qt + 1) * P, :], ob[:])

    # ---- hand-woven emission order ----
    # One ~2.3us filler unit after each energy group paces E emission at the
    # ACT exp drain rate (~2.7us/group). V runs in windows 0-1 so pv(ft)
    # lands at window ft+2 (PT in flight <= 34, pool 35). The tail orders
    # pv(3) by q-half so OUT chains for qt 0-3 overlap pv(3,*,1).
    def K(ft, kh):
        return lambda: proj_half(ft, kh, wk_t, xk_t, KT, bk_t)

    def Q(ft, kh):
        return lambda: proj_half(ft, kh, wq_t, xq_t, QT, bq_t)

    def seq(*fs):
        return lambda: [f() for f in fs]

    def MM(ft, hh, qh):
        return lambda: pv_mm(ft, hh, qh)

    def FIN(ft, hh, qh):
        return lambda: pv_fin(ft, hh, qh)

    for f in (K(0, 0), K(0, 1), Q(0, 0), Q(0, 1)):
        f()
    fillers = [
        K(1, 0), K(1, 1), Q(1, 0), Q(1, 1),
    ] + [lambda tt=tt: vproj(tt) for tt in range(NKT)] + [
        K(2, 0), K(2, 1), Q(2, 0), Q(2, 1),
        lambda: pv(0, 0, 0), lambda: pv(0, 0, 1),
        lambda: pv(0, 1, 0), lambda: pv(0, 1, 1),
        K(3, 0), K(3, 1), Q(3, 0), Q(3, 1),
        lambda: pv(1, 0, 0), lambda: pv(1, 0, 1),
        lambda: pv(1, 1, 0), lambda: pv(1, 1, 1),
        lambda: pv(2, 0, 0), lambda: pv(2, 0, 1),
        lambda: pv(2, 1, 0), lambda: pv(2, 1, 1),
    ]
    fi = iter(fillers)
    for ft in range(FT):
        for kt in range(NKT):
            energy(ft, kt)
            f = next(fi, None)
            if f is not None:
                f()
    if emit_pv3:
        pv(3, 0, 0)
        pv(3, 1, 0)
    if emit_out:
        for qt in range(4):
            outp(qt)
            if qt < 2 and emit_pv3:
                pv(3, qt, 1)
        for qt in range(4, ET):
            outp(qt)
    elif emit_pv3:
        pv(3, 0, 1)
        pv(3, 1, 1)
    if refill:
        h["emit_loads"]()


def build_nc(repeats=1, hw_loop=0, refill=True, emit_out=True, emit_pv3=True):
    nc = bacc.Bacc()
    t = _declare(nc)
    with tile.TileContext(nc) as tc:
        h = _emit_prelude(nc, tc, t)
        if hw_loop:
            with tc.For_i(0, hw_loop, 1, staggered_reset=True):
                _emit_compute(nc, tc, t, h, refill=refill,
                              emit_out=emit_out, emit_pv3=emit_pv3)
        else:
            for _ in range(repeats):
                _emit_compute(nc, tc, t, h, refill=False)
        h["ctx"].close()
    nc.finalize()
    return nc


_NC = None


def _get_nc():
    global _NC
    if _NC is None:
        _NC = build_nc()
    return _NC


def _prep_in_maps(value, key_in, query, mask, Wq, bq, Wk, bk, Wv, bv, Wo, bo):
    f = np.float32
    value = np.asarray(value, f)
    key_in = np.asarray(key_in, f)
    query = np.asarray(query, f)
    mask = np.asarray(mask)
    Wq = np.asarray(Wq, f); bq = np.asarray(bq, f)
    Wk = np.asarray(Wk, f); bk = np.asarray(bk, f)
    Wv = np.asarray(Wv, f); bv = np.asarray(bv, f)
    Wo = np.asarray(Wo, f); bo = np.asarray(bo, f)

    s = f(1.0 / np.sqrt(E))
    wqT = (Wq.T * s).astype(NPBF16)
    wkT = Wk.T.astype(NPBF16)
    wvT = Wv.T.astype(NPBF16)
    woT = Wo.T.astype(NPBF16)
    bo_eff = bo + Wo @ bv
    bo_full = np.ascontiguousarray(
        np.broadcast_to(bo_eff, (P, E))).astype(NPBF16)
    bo_zero = np.zeros((P, E), NPBF16)
    ones_t = np.ones((P, D), NPBF16)

    xT = {}
    for b in range(B):
        xT[b] = (
            np.ascontiguousarray(query[b].astype(NPBF16).T),
            np.ascontiguousarray(key_in[b].astype(NPBF16).T),
            np.ascontiguousarray(value[b].astype(NPBF16).T),
        )

    in_maps = []
    for c in range(N_CORES):
        b, g = c // 2, c % 2
        cols = slice(g * EH, (g + 1) * EH)
        mrow = mask[b, 0, 0, :]
        mb = np.where(mrow == 0, f(-50.0), f(0.0)).astype(f)
        xq_b, xk_b, xv_b = xT[b]
        in_maps.append({
            "xqT": xq_b, "xkT": xk_b, "xvT": xv_b,
            "wq": np.ascontiguousarray(wqT[:, cols]),
            "wk": np.ascontiguousarray(wkT[:, cols]),
            "wv": np.ascontiguousarray(wvT[:, cols]),
            "wo": np.ascontiguousarray(woT[cols, :]),
            "bq": np.ascontiguousarray((bq[cols] * s).reshape(FT, P).T),
            "bk": np.ascontiguousarray(bk[cols].reshape(FT, P).T),
            "maskb": np.ascontiguousarray(mb.reshape(NKT, P).T),
            "boeff": bo_full if g == 0 else bo_zero,
            "ones": ones_t,
        })
    return in_maps


def _assemble(results):
    out = np.empty((B, S, E), np.float32)
    for b in range(B):
        out[b] = results[2 * b]["out"] + results[2 * b + 1]["out"]
    return out


def kernel(value, key_in, query, mask, Wq, bq, Wk, bk, Wv, bv, Wo, bo):
    nc = _get_nc()
    in_maps = _prep_in_maps(value, key_in, query, mask,
                            Wq, bq, Wk, bk, Wv, bv, Wo, bo)
    r = run_bass_kernel_spmd(nc, in_maps, list(range(N_CORES)))
    return _assemble(r.results)


def kernel_traced(value, key_in, query, mask, Wq, bq, Wk, bk, Wv, bv, Wo, bo,
                  **trace_kwargs):
    """Like kernel() but returns (output, BassKernelResults) with profiling."""
    nc = _get_nc()
    in_maps = _prep_in_maps(value, key_in, query, mask,
                            Wq, bq, Wk, bk, Wv, bv, Wo, bo)
    r = run_bass_kernel_spmd(nc, in_maps, list(range(N_CORES)), trace=True,
                             **trace_kwargs)
    return _assemble(r.results), r



# revision 24
# speedup vs baseline: 1.0305x; 1.0029x over previous
"""Multi-head attention Trainium2 kernel (nn_MultiHeadAttention_7035156430929).

B=4, S=1024, E=1024, H=16, D=64. Sharding: 8 cores = 4 batches x 2
head-groups (tensor parallel over heads, per the hint). Each core computes 8
heads for all 1024 queries/keys of its batch: Wq/Wk/Wv column-sliced (512
features), Wo row-sliced, giving a PARTIAL output [1024, 1024] per core; the
two partials of a batch are summed on the host (the "all-reduce after fc_out"
done host-side since core outputs are gathered anyway).

Everything on device is bf16 except PSUM accumulation, softmax denominators,
biases, and the output partials (fp32). Measured-HW design points:
  - matmul N=512 ~285 ns; K=64 pairs on PE row-groups (tile_position 0/64)
    run concurrently (~308 ns/pair) -> energy QK^T paired across head pairs.
  - exp on ACT: [128,1024] = 1324 ns; energy PSUM tiles are [128,1024]
    (2 banks) so each (head, kt) needs ONE activation.
  - PE emission is hand-woven: energy groups are spaced between projection /
    PV / output chains so the ACT engine (85 us of exp) hides under the PE
    stream (~128 us) and energy PSUM slots (3x2 banks) never block.
  - In the For_i timing loop (whose per-iteration all-engine barrier
    serializes bodies), input tiles persist and each iteration RE-LOADS them
    at its tail, so the HBM traffic stays per-iteration but the next body's
    projections start right after the barrier (no DMA lead-in). The loop
    uses staggered_reset=True (-5us/iter of barrier/sem-reset tax).
  - outp emits both 512-wide output halves into one [128,1024] psum tile
    (8 MMs, one DVE bias-add, one DMA store per query block): ~-8us vs
    per-half chains.
1/sqrt(E) folds into Wq/bq; bv folds into the g=0 core's output bias
(softmax rows sum to 1); key-padding mask becomes an additive per-key bias
(-50) inside the exp activation.
"""
import sys

sys.path.insert(0, "/opt/trn_rl_repo")

from contextlib import ExitStack

import numpy as np
import ml_dtypes

import concourse.bacc as bacc
import concourse.tile as tile
from concourse import mybir
from concourse.bass_utils import run_bass_kernel_spmd

B, S, E, H, D = 4, 1024, 1024, 16, 64
P = 128
N_CORES = 8
HC = 8             # heads per core
EH = 512           # features per core
FT = 4             # feature tiles of 128 (2 heads each)
ET = 8             # contraction tiles over E
NKT = 8            # key-token tiles
DP1 = D + 1        # V columns per head incl. ones column
F32 = mybir.dt.float32
BF16 = mybir.dt.bfloat16
AF = mybir.ActivationFunctionType
NPBF16 = ml_dtypes.bfloat16


def _declare(nc):
    dp = nc.declare_dram_parameter
    t = {}
    t["xqT"] = dp("xqT", [E, S], BF16, isOutput=False)   # query[b].T
    t["xkT"] = dp("xkT", [E, S], BF16, isOutput=False)
    t["xvT"] = dp("xvT", [E, S], BF16, isOutput=False)
    t["wq"] = dp("wq", [E, EH], BF16, isOutput=False)    # Wq.T col-slice, *s
    t["wk"] = dp("wk", [E, EH], BF16, isOutput=False)
    t["wv"] = dp("wv", [E, EH], BF16, isOutput=False)
    t["wo"] = dp("wo", [EH, E], BF16, isOutput=False)    # Wo.T row-slice
    t["bq"] = dp("bq", [P, FT], F32, isOutput=False)
    t["bk"] = dp("bk", [P, FT], F32, isOutput=False)
    t["maskb"] = dp("maskb", [P, NKT], F32, isOutput=False)  # 0 or -50
    t["boeff"] = dp("boeff", [P, E], BF16, isOutput=False)   # row-replicated
    t["ones"] = dp("ones", [P, D], BF16, isOutput=False)
    t["out"] = dp("out", [S, E], F32, isOutput=True)         # partial
    t["out2"] = dp("out2", [S, E], F32, isOutput=True)       # timing-loop pair
    return t


def _emit_prelude(nc, tc, t):
    """Create all pools/tiles and emit the initial input loads. Returns a
    state dict shared by every _emit_compute call (tiles persist across
    For_i iterations; each iteration refills them at its tail so the next
    iteration's projections start immediately after the loop barrier)."""
    xqT, xkT, xvT = t["xqT"], t["xkT"], t["xvT"]
    wq, wk, wv, wo = t["wq"], t["wk"], t["wv"], t["wo"]
    bq, bk, maskb, boeff = t["bq"], t["bk"], t["maskb"], t["boeff"]
    ones = t["ones"]

    ctx = ExitStack()
    const = ctx.enter_context(tc.tile_pool(name="const", bufs=1))
    bq_t = const.tile([P, FT], F32, tag="bq", name="bq")
    bk_t = const.tile([P, FT], F32, tag="bk", name="bk")
    mb_t = const.tile([P, NKT], F32, tag="mb", name="mb")
    bo_t = const.tile([P, E], BF16, tag="bo", name="bo")
    on_t = const.tile([1, D], BF16, tag="on", name="on")
    nc.gpsimd.dma_start(bq_t[:], bq.ap()[:])
    nc.gpsimd.dma_start(bk_t[:], bk.ap()[:])
    nc.gpsimd.dma_start(mb_t[:], maskb.ap()[:])
    nc.gpsimd.dma_start(bo_t[:], boeff.ap()[:])
    nc.gpsimd.dma_start(on_t[:], ones.ap()[0:1, :])

    # Persistent per-iteration intermediates
    kt_pool = ctx.enter_context(tc.tile_pool(name="ktp", bufs=FT))
    qt_pool = ctx.enter_context(tc.tile_pool(name="qtp", bufs=FT))
    va_pool = ctx.enter_context(tc.tile_pool(name="vap", bufs=NKT))
    atn_pool = ctx.enter_context(tc.tile_pool(name="atn", bufs=FT))
    pt_pool = ctx.enter_context(tc.tile_pool(name="pt", bufs=35))
    nrm_pool = ctx.enter_context(tc.tile_pool(name="nrm", bufs=2))
    ob_pool = ctx.enter_context(tc.tile_pool(name="ob", bufs=2))
    wo_pool = ctx.enter_context(tc.tile_pool(name="wop", bufs=1))
    ps2 = ctx.enter_context(tc.tile_pool(name="ps2", bufs=3, space="PSUM"))
    ps1 = ctx.enter_context(tc.tile_pool(name="ps1", bufs=2, space="PSUM"))

    KT = [kt_pool.tile([P, S], BF16, tag="kt", name="kt") for _ in range(FT)]
    QT = [qt_pool.tile([P, S], BF16, tag="qt", name="qt") for _ in range(FT)]
    VA = [va_pool.tile([P, HC * DP1], BF16, tag="va", name="va")
          for _ in range(NKT)]
    ATN = [atn_pool.tile([P, S], BF16, tag="at", name="at") for _ in range(FT)]
    PT = {}

    # Streamed activations / weights (per-et tiles; first chain starts after
    # just its first two small DMAs land)
    xk_p = ctx.enter_context(tc.tile_pool(name="xk", bufs=ET))
    xq_p = ctx.enter_context(tc.tile_pool(name="xq", bufs=ET))
    wk_p = ctx.enter_context(tc.tile_pool(name="wk", bufs=FT))
    wq_p = ctx.enter_context(tc.tile_pool(name="wq", bufs=FT))
    xv_p = ctx.enter_context(tc.tile_pool(name="xv", bufs=ET))
    wv_p = ctx.enter_context(tc.tile_pool(name="wv", bufs=ET))

    xk_t = [xk_p.tile([P, S], BF16, tag="xk", name="xk") for _ in range(ET)]
    xq_t = [xq_p.tile([P, S], BF16, tag="xq", name="xq") for _ in range(ET)]
    wk_t = [wk_p.tile([P, ET * P], BF16, tag="wk", name="wk")
            for _ in range(FT)]
    wq_t = [wq_p.tile([P, ET * P], BF16, tag="wq", name="wq")
            for _ in range(FT)]
    xv_t = [xv_p.tile([P, S], BF16, tag="xv", name="xv") for _ in range(ET)]
    wv_t = [wv_p.tile([P, EH], BF16, tag="wv", name="wv") for _ in range(ET)]
    wo_t = wo_pool.tile([P, FT * E], BF16, tag="wo", name="wo")

    def _wf_load(tt, dram, ft):
        nc.sync.dma_start(
            tt[:].rearrange("p (a s) -> p a s", s=P),
            dram.ap()[:, ft * P:(ft + 1) * P].rearrange(
                "(a p) s -> p a s", p=P))

    def emit_loads():
        for et in range(ET):
            nc.sync.dma_start(xk_t[et][:], xkT.ap()[et * P:(et + 1) * P, :])
        _wf_load(wk_t[0], wk, 0)
        for et in range(ET):
            nc.sync.dma_start(xq_t[et][:], xqT.ap()[et * P:(et + 1) * P, :])
        _wf_load(wq_t[0], wq, 0)
        for ft in range(1, FT):
            _wf_load(wk_t[ft], wk, ft)
            _wf_load(wq_t[ft], wq, ft)
        for et in range(ET):
            nc.sync.dma_start(xv_t[et][:], xvT.ap()[et * P:(et + 1) * P, :])
            nc.sync.dma_start(wv_t[et][:], wv.ap()[et * P:(et + 1) * P, :])
        nc.sync.dma_start(
            wo_t[:].rearrange("p (a s) -> p a s", s=E),
            wo.ap()[:].rearrange("(a p) s -> p a s", p=P))

    emit_loads()
    for tt in range(NKT):
        ones3 = VA[tt][:].rearrange("p (h c) -> p h c", c=DP1)[:, :, D:DP1]
        nc.gpsimd.dma_start(
            ones3, ones.ap()[:, 0:HC].rearrange("p (h c) -> p h c", c=1))

    return dict(ctx=ctx, KT=KT, QT=QT, VA=VA, ATN=ATN, on_t=on_t,
                kt_b=(bq_t, bk_t, mb_t, bo_t),
                xk_t=xk_t, xq_t=xq_t, wk_t=wk_t, wq_t=wq_t,
                xv_t=xv_t, wv_t=wv_t, wo_t=wo_t,
                ps2=ps2, ps1=ps1, pt_pool=pt_pool, nrm_pool=nrm_pool,
                ob_pool=ob_pool, emit_loads=emit_loads)


def _emit_compute(nc, tc, t, h, out_key="out", refill=False,
                  emit_out=True, emit_pv3=True):
    out = t[out_key]
    KT, QT, VA, ATN = h["KT"], h["QT"], h["VA"], h["ATN"]
    bq_t, bk_t, mb_t, bo_t = h["kt_b"]
    xk_t, xq_t, wk_t, wq_t = h["xk_t"], h["xq_t"], h["wk_t"], h["wq_t"]
    xv_t, wv_t, wo_t = h["xv_t"], h["wv_t"], h["wo_t"]
    ps2, ps1 = h["ps2"], h["ps1"]
    pt_pool, nrm_pool, ob_pool = h["pt_pool"], h["nrm_pool"], h["ob_pool"]
    PT = {}

    def proj_half(ft, kh, w_t, x_t, dst, bias_t):
        # Self-contained half-chain (8 mm + bias-add) in a 1-bank ps1 tile.
        # Keeping proj OFF the ps2 pool preserves the energy/exp 3-slot
        # rotation (energy(g+1) waits one exp, not two).
        ps = ps1.tile([P, 512], F32, tag="p1", name="p1")
        for et in range(ET):
            nc.tensor.matmul(
                ps[:], w_t[ft][:, et * P:(et + 1) * P],
                x_t[et][:, kh * 512:(kh + 1) * 512],
                start=(et == 0), stop=(et == ET - 1))
        nc.vector.tensor_scalar_add(dst[ft][:, kh * 512:(kh + 1) * 512],
                                    ps[:], bias_t[:, ft:ft + 1])

    def vproj(tt):
        ps = ps1.tile([P, 512], F32, tag="p1", name="p1")
        for et in range(ET):
            nc.tensor.matmul(
                ps[:], xv_t[et][:, tt * P:(tt + 1) * P], wv_t[et][:],
                start=(et == 0), stop=(et == ET - 1))
        va3 = VA[tt][:].rearrange("p (h c) -> p h c", c=DP1)[:, :, 0:D]
        ps3 = ps[:].rearrange("p (h c) -> p h c", c=D)
        nc.vector.tensor_copy(va3, ps3)

    def energy(ft, kt):
        pe_h = [ps2.tile([P, S], F32, tag="p2", name="p2") for _ in range(2)]
        for qh in range(2):
            for hh in range(2):
                hp = hh * D
                nc.tensor.matmul(
                    pe_h[hh][:, qh * 512:(qh + 1) * 512],
                    KT[ft][hp:hp + D, kt * P:(kt + 1) * P],
                    QT[ft][hp:hp + D, qh * 512:(qh + 1) * 512],
                    start=True, stop=True, tile_position=(hp, 0))
        for hh in range(2):
            pt = pt_pool.tile([P, S], BF16, tag="pt", name="pt")
            nc.scalar.activation(pt[:], pe_h[hh][:], AF.Exp,
                                 bias=mb_t[:, kt:kt + 1])
            PT[(ft, hh, kt)] = pt

    pv_live = {}

    def pv_mm(ft, hh, qh):
        # PV accumulation + reciprocal; the broadcast+multiply finisher is
        # deferred (pv_fin) so its Pool/DVE latency stays off the window
        # critical path.
        hc = ft * 2 + hh
        ps = ps1.tile([DP1, 512], F32, tag="p1", name="p1")
        for kt in range(NKT):
            nc.tensor.matmul(
                ps[:], VA[kt][:, hc * DP1:(hc + 1) * DP1],
                PT[(ft, hh, kt)][:, qh * 512:(qh + 1) * 512],
                start=(kt == 0), stop=(kt == NKT - 1))
        rec = nrm_pool.tile([1, 512], F32, tag="rec", name="rec")
        nc.vector.reciprocal(rec[0:1, :], ps[D:DP1, :])
        pv_live[(ft, hh, qh)] = (ps, rec)

    def pv_fin(ft, hh, qh):
        ps, rec = pv_live.pop((ft, hh, qh))
        bc = nrm_pool.tile([D, 512], F32, tag="bc", name="bc")
        nc.gpsimd.partition_broadcast(bc[0:D, :], rec[0:1, :])
        nc.vector.tensor_mul(
            ATN[ft][hh * D:(hh + 1) * D, qh * 512:(qh + 1) * 512],
            ps[0:D, :], bc[0:D, :])

    def pv(ft, hh, qh):
        pv_mm(ft, hh, qh)
        pv_fin(ft, hh, qh)

    def outp(qt):
        # Both 512-wide output halves accumulate into one [128,1024] psum
        # tile: 8 MMs, then a single DVE bias-add and a single DMA store.
        ps = ps2.tile([P, S], F32, tag="p2", name="p2")
        for ft in range(FT):
            for eb in range(2):
                nc.tensor.matmul(
                    ps[:, eb * 512:(eb + 1) * 512],
                    ATN[ft][:, qt * P:(qt + 1) * P],
                    wo_t[:, ft * E + eb * 512:ft * E + (eb + 1) * 512],
                    start=(ft == 0), stop=(ft == FT - 1))
        ob = ob_pool.tile([P, S], F32, tag="ob", name="ob")
        nc.vector.tensor_add(ob[:], ps[:], bo_t[:])
        nc.scalar.dma_start(out.ap()[qt * P:(qt + 1) * P, :], ob[:])

    # ---- hand-woven emission order ----
    # One ~2.3us filler unit after each energy group paces E emission at the
    # ACT exp drain rate (~2.7us/group). V runs in windows 0-1 so pv(ft)
    # lands at window ft+2 (PT in flight <= 34, pool 35). The tail orders
    # pv(3) by q-half so OUT chains for qt 0-3 overlap pv(3,*,1).
    def K(ft, kh):
        return lambda: proj_half(ft, kh, wk_t, xk_t, KT, bk_t)

    def Q(ft, kh):
        return lambda: proj_half(ft, kh, wq_t, xq_t, QT, bq_t)

    def seq(*fs):
        return lambda: [f() for f in fs]

    def MM(ft, hh, qh):
        return lambda: pv_mm(ft, hh, qh)

    def FIN(ft, hh, qh):
        return lambda: pv_fin(ft, hh, qh)

    for f in (K(0, 0), K(0, 1), Q(0, 0), Q(0, 1)):
        f()
    fillers = [
        K(1, 0), K(1, 1), Q(1, 0), Q(1, 1),
    ] + [lambda tt=tt: vproj(tt) for tt in range(NKT)] + [
        K(2, 0), K(2, 1), Q(2, 0), Q(2, 1),
        lambda: pv(0, 0, 0), lambda: pv(0, 0, 1),
        lambda: pv(0, 1, 0), lambda: pv(0, 1, 1),
        K(3, 0), K(3, 1), Q(3, 0), Q(3, 1),
        lambda: pv(1, 0, 0), lambda: pv(1, 0, 1),
        lambda: pv(1, 1, 0), lambda: pv(1, 1, 1),
        lambda: pv(2, 0, 0), lambda: pv(2, 0, 1),
        lambda: pv(2, 1, 0), lambda: pv(2, 1, 1),
    ]
    fi = iter(fillers)
    for ft in range(FT):
        for kt in range(NKT):
            energy(ft, kt)
            f = next(fi, None)
            if f is not None:
                f()
    if emit_pv3:
        pv(3, 0, 0)
        pv(3, 1, 0)
    if emit_out:
        for qt in range(4):
            outp(qt)
            if qt < 2 and emit_pv3:
                pv(3, qt, 1)
        for qt in range(4, ET):
            outp(qt)
    elif emit_pv3:
        pv(3, 0, 1)
        pv(3, 1, 1)
    if refill:
        h["emit_loads"]()


def build_nc(repeats=1, hw_loop=0, refill=True, emit_out=True, emit_pv3=True):
    nc = bacc.Bacc()
    t = _declare(nc)
    with tile.TileContext(nc) as tc:
        h = _emit_prelude(nc, tc, t)
        if hw_loop:
            with tc.For_i(0, hw_loop, 1, staggered_reset=True):
                _emit_compute(nc, tc, t, h, refill=refill,
                              emit_out=emit_out, emit_pv3=emit_pv3)
        else:
            for _ in range(repeats):
                _emit_compute(nc, tc, t, h, refill=False)
        h["ctx"].close()
    nc.finalize()
    return nc


_NC = None


def _get_nc():
    global _NC
    if _NC is None:
        _NC = build_nc()
    return _NC


def _prep_in_maps(value, key_in, query, mask, Wq, bq, Wk, bk, Wv, bv, Wo, bo):
    f = np.float32
    value = np.asarray(value, f)
    key_in = np.asarray(key_in, f)
    query = np.asarray(query, f)
    mask = np.asarray(mask)
    Wq = np.asarray(Wq, f); bq = np.asarray(bq, f)
    Wk = np.asarray(Wk, f); bk = np.asarray(bk, f)
    Wv = np.asarray(Wv, f); bv = np.asarray(bv, f)
    Wo = np.asarray(Wo, f); bo = np.asarray(bo, f)

    s = f(1.0 / np.sqrt(E))
    wqT = (Wq.T * s).astype(NPBF16)
    wkT = Wk.T.astype(NPBF16)
    wvT = Wv.T.astype(NPBF16)
    woT = Wo.T.astype(NPBF16)
    bo_eff = bo + Wo @ bv
    bo_full = np.ascontiguousarray(
        np.broadcast_to(bo_eff, (P, E))).astype(NPBF16)
    bo_zero = np.zeros((P, E), NPBF16)
    ones_t = np.ones((P, D), NPBF16)

    xT = {}
    for b in range(B):
        xT[b] = (
            np.ascontiguousarray(query[b].astype(NPBF16).T),
            np.ascontiguousarray(key_in[b].astype(NPBF16).T),
            np.ascontiguousarray(value[b].astype(NPBF16).T),
        )

    in_maps = []
    for c in range(N_CORES):
        b, g = c // 2, c % 2
        cols = slice(g * EH, (g + 1) * EH)
        mrow = mask[b, 0, 0, :]
        mb = np.where(mrow == 0, f(-50.0), f(0.0)).astype(f)
        xq_b, xk_b, xv_b = xT[b]
        in_maps.append({
            "xqT": xq_b, "xkT": xk_b, "xvT": xv_b,
            "wq": np.ascontiguousarray(wqT[:, cols]),
            "wk": np.ascontiguousarray(wkT[:, cols]),
            "wv": np.ascontiguousarray(wvT[:, cols]),
            "wo": np.ascontiguousarray(woT[cols, :]),
            "bq": np.ascontiguousarray((bq[cols] * s).reshape(FT, P).T),
            "bk": np.ascontiguousarray(bk[cols].reshape(FT, P).T),
            "maskb": np.ascontiguousarray(mb.reshape(NKT, P).T),
            "boeff": bo_full if g == 0 else bo_zero,
            "ones": ones_t,
        })
    return in_maps


def _assemble(results):
    out = np.empty((B, S, E), np.float32)
    for b in range(B):
        out[b] = results[2 * b]["out"] + results[2 * b + 1]["out"]
    return out


def kernel(value, key_in, query, mask, Wq, bq, Wk, bk, Wv, bv, Wo, bo):
    nc = _get_nc()
    in_maps = _prep_in_maps(value, key_in, query, mask,
                            Wq, bq, Wk, bk, Wv, bv, Wo, bo)
    r = run_bass_kernel_spmd(nc, in_maps, list(range(N_CORES)))
    return _assemble(r.results)


def kernel_traced(value, key_in, query, mask, Wq, bq, Wk, bk, Wv, bv, Wo, bo,
                  **trace_kwargs):
    """Like kernel() but returns (output, BassKernelResults) with profiling."""
    nc = _get_nc()
    in_maps = _prep_in_maps(value, key_in, query, mask,
                            Wq, bq, Wk, bk, Wv, bv, Wo, bo)
    r = run_bass_kernel_spmd(nc, in_maps, list(range(N_CORES)), trace=True,
                             **trace_kwargs)
    return _assemble(r.results), r

